# revision 17
# baseline (speedup 1.0000x reference)
"""Trainium2 Bass kernel for nn_ConditionalNSF (conditional neural spline flow NLL).

Strategy: pure data parallel over 8 NeuronCores (2048 rows each).
 - Host: tiny param prep (MADE masks, LU-fold into MADE layer 0, final-layer
   column reorder + spline boundary-derivative bias trick), final scalar mean.
 - Device, per core:
   * feature-major encoder MLP (activations [feat, batch]) with float32r
     matmuls at N=512; LayerNorm stats via PE ones-matmuls + ACT Square.
   * 4 flow layers: LU step folded into an 8x8 matmul; MADE residual MLP
     feature-major; final MADE matmul emits batch-major spline params
     (swapped operands, N=256) straight into an Exp activation.
   * rational-quadratic spline fully batch-major: cumsum via strided adds,
     bin search + one-hot gathers on DVE/GPSIMD, exp/ln/softplus on ACT.
   * per-row logq accumulated on chip, [128,16] DMA'd out per core.
"""
import sys

sys.path.insert(0, "/opt/trn_rl_repo")

import numpy as np

import concourse.bass as bass
import concourse.bacc as bacc
import concourse.tile as tile
from concourse import mybir
from concourse.bass_utils import run_bass_kernel_spmd
from concourse.masks import make_identity

AF = mybir.ActivationFunctionType
ALU = mybir.AluOpType
F32 = mybir.dt.float32
BF = mybir.dt.bfloat16

# ---- problem dims (hardcoded) ----
B = 16384
NCORES = 8
BC = B // NCORES          # 2048 rows per core
NBLK = 4                  # batch blocks of 512 per core
BLK = 512
NCH = 16                  # chunks of 128 per core
SD, AD, CTX, HID = 128, 32, 512, 512
FD, H, NB, K, L = 8, 256, 2, 8, 4
M = 3 * K - 1
TB = 6.0
MBW = MBH = MD = 1e-3
LN_EPS = 1e-5
SQH = float(np.sqrt(H))
CPAD = float(np.log(np.expm1(1.0 - MD)))
CW_SCALE = 2.0 * TB * (1.0 - MBW * K)   # cum * rec * this + kbias


def _masks():
    in_deg = np.arange(1, FD + 1)
    hid_deg = np.arange(H) % max(1, FD - 1) + min(1, FD - 1)
    m0 = (hid_deg[:, None] >= in_deg[None, :]).astype(np.float32)
    mh = (hid_deg[:, None] >= hid_deg[None, :]).astype(np.float32)
    out_deg = np.repeat(in_deg, M)
    mf = (out_deg[:, None] > hid_deg[None, :]).astype(np.float32)
    return m0, mh, mf


def _softplus(x):
    return np.log1p(np.exp(x))


def _f32(x):
    return np.ascontiguousarray(np.asarray(x, np.float32))


def _bf16(x):
    import ml_dtypes
    return np.ascontiguousarray(np.asarray(x, np.float32).astype(ml_dtypes.bfloat16))


def prep_host(state, action, x_pad, enc_params, flow_params, perms):
    """Returns (shared_inputs dict, per_core list of dicts, host_const float)."""
    p = {k: _f32(v) for k, v in enc_params.items()}
    fp = {k: _f32(v) for k, v in flow_params.items()}
    perms = np.asarray(perms)
    m0, mh, mf = _masks()

    sh = {}
    # --- encoder weights, transposed [D_in, D_out] ---
    sh["wsT"] = _f32(p["Ws"].T)                               # [128, 512]
    sh["waT"] = _f32(p["Wa"].T)                               # [32, 512]
    sh["wf1T"] = _f32(p["Wf1"].T.reshape(8, 128, 512).transpose(1, 0, 2)
                      .reshape(128, 8 * 512))                 # [128, kt*512]
    for nm, tag in (("Wf2", "wf2T"), ("Wo1", "wo1T"), ("Wo2", "wo2T")):
        sh[tag] = _f32(p[nm].T.reshape(4, 128, 512).transpose(1, 0, 2)
                       .reshape(128, 4 * 512))
    # encoder biases [128, 6, 4] (slot l, chunk m)
    encbp = np.zeros((128, 6, 4), np.float32)
    for l, nm in enumerate(["bs", "ba", "bf1", "bf2", "bo1", "bo2"]):
        encbp[:, l, :] = p[nm].reshape(4, 128).T
    sh["encbp"] = _f32(encbp.reshape(128, 24))
    encg = np.zeros((128, 5, 4), np.float32)
    encbg = np.zeros((128, 5, 4), np.float32)
    for l, (g, bg) in enumerate([("gs", "bgs"), ("ga", "bga"), ("gf1", "bgf1"),
                                 ("gf2", "bgf2"), ("go1", "bgo1")]):
        encg[:, l, :] = p[g].reshape(4, 128).T
        encbg[:, l, :] = p[bg].reshape(4, 128).T
    sh["encg"] = _f32(encg.reshape(128, 20))
    sh["encbg"] = _f32(encbg.reshape(128, 20))

    # --- flow prep ---
    A = np.zeros((L, FD, FD), np.float32)
    ld_host = 0.0
    for i in range(L):
        Pm = np.zeros((FD, FD), np.float32)
        Pm[perms[i], np.arange(FD)] = 1.0
        lw = np.tril(fp["lu_L"][i], -1) + np.eye(FD, dtype=np.float32)
        udiag = _softplus(fp["lu_d"][i]) + 1e-3
        up = np.triu(fp["lu_U"][i], 1) + np.diag(udiag)
        A[i] = Pm @ up.T @ lw.T
        ld_host += float(np.sum(np.log(udiag)))
    sh["a_sb"] = _f32(A.transpose(1, 0, 2).reshape(FD, L * FD))   # [8, L*8]
    sh["lub"] = _f32(fp["lu_b"].T)                                # [8, L]

    W0m = fp["W0"] * m0[None]
    w0a = np.stack([A[i] @ W0m[i].T for i in range(L)])           # [L, 8, H]
    sh["w0a"] = _f32(w0a.transpose(1, 0, 2).reshape(FD, L * H))   # [8, L*256]
    bias0 = np.stack([fp["lu_b"][i] @ W0m[i].T + fp["b0"][i] + fp["bc0"][i]
                      for i in range(L)])                         # [L, H]

    # per-partition bias pack [128, L, 14]:
    # slots: 0-1 bias0(m), then per j (j*6): 2+6j bb1(m), 4+6j bb2(m)(UNUSED,
    # bb2 folded into mt STT), ... keep simple: 0-1 bias0, 2-3 bb1 j0,
    # 4-5 bb2 j0, 6-7 bcb j0, 8-9 bb1 j1, 10-11 bb2 j1, 12-13 bcb j1
    fbp = np.zeros((128, L, 14), np.float32)
    for i in range(L):
        fbp[:, i, 0:2] = bias0[i].reshape(2, 128).T
        for j in range(NB):
            fbp[:, i, 2 + 6 * j:4 + 6 * j] = fp["bb1"][i, j].reshape(2, 128).T
            fbp[:, i, 4 + 6 * j:6 + 6 * j] = fp["bb2"][i, j].reshape(2, 128).T
            fbp[:, i, 6 + 6 * j:8 + 6 * j] = fp["bcb"][i, j].reshape(2, 128).T
    sh["fbp"] = _f32(fbp.reshape(128, L * 14))

    # final-layer rhs [H, 256] per layer: cols [UW(64)|UH(64)|D0(64)|D1(64)]
    Wfm = fp["Wf"] * mf[None]
    bf = fp["bf"]
    wft = np.zeros((L, H, 256), np.float32)
    bfr = np.zeros((L, 256), np.float32)
    for i in range(L):
        for f in range(FD):
            for k in range(K):
                wft[i, :, f * K + k] = Wfm[i, f * M + k] / SQH
                bfr[i, f * K + k] = bf[i, f * M + k] / SQH
                wft[i, :, 64 + f * K + k] = Wfm[i, f * M + K + k] / SQH
                bfr[i, 64 + f * K + k] = bf[i, f * M + K + k] / SQH
                if k == 0:
                    bfr[i, 128 + f * K] = CPAD
                else:
                    wft[i, :, 128 + f * K + k] = Wfm[i, f * M + 2 * K + k - 1]
                    bfr[i, 128 + f * K + k] = bf[i, f * M + 2 * K + k - 1]
                if k == K - 1:
                    bfr[i, 192 + f * K + k] = CPAD
                else:
                    wft[i, :, 192 + f * K + k] = Wfm[i, f * M + 2 * K + k]
                    bfr[i, 192 + f * K + k] = bf[i, f * M + 2 * K + k]
    sh["bfr"] = _f32(bfr.reshape(1, L * 256))

    # streamed per-layer weight pack [L, 128, 5632]:
    # cols: wc0 (kt4,256)=0:1024 | wb1 (j2,kt2,256)=1024:2048 |
    #       wb2 (j2,kt2,256)=2048:3072 | wcb (j2,kt4,256)=3072:5120 |
    #       wft (kt2,256)=5120:5632
    flw = np.zeros((L, 128, 5632), np.float32)
    for i in range(L):
        wc0T = fp["Wc0"][i].T            # [CTX, H]
        flw[i, :, 0:1024] = wc0T.reshape(4, 128, 256).transpose(1, 0, 2).reshape(128, 1024)
        for j in range(NB):
            b1 = (fp["Wb1"][i, j] * mh).T    # [H, H]
            b2 = (fp["Wb2"][i, j] * mh).T
            cb = fp["Wcb"][i, j].T           # [CTX, H]
            flw[i, :, 1024 + 512 * j:1024 + 512 * (j + 1)] = \
                b1.reshape(2, 128, 256).transpose(1, 0, 2).reshape(128, 512)
            flw[i, :, 2048 + 512 * j:2048 + 512 * (j + 1)] = \
                b2.reshape(2, 128, 256).transpose(1, 0, 2).reshape(128, 512)
            flw[i, :, 3072 + 1024 * j:3072 + 1024 * (j + 1)] = \
                cb.reshape(4, 128, 256).transpose(1, 0, 2).reshape(128, 1024)
        flw[i, :, 5120:5632] = wft[i].reshape(2, 128, 256).transpose(1, 0, 2).reshape(128, 512)
    sh["flw"] = _f32(flw)

    # spline consts
    kk = np.arange(1, K + 1, dtype=np.float32)
    sh["kbias"] = _f32(np.broadcast_to(2 * TB * MBW * kk - TB, (128, K)))
    sh["kiota"] = _f32(np.broadcast_to(np.arange(K, dtype=np.float32), (128, K)))

    for k in ["a_sb", "w0a", "bfr", "flw"]:
        sh[k] = _bf16(sh[k])

    # single packed bf16 tensor for ALL encoder-phase DMAs (one DMA proc ->
    # small released-zone dep sets). col layout:
    # state 0:2048 | action 2048:4096 (rows<32) | wsT 4096:4608 |
    # waT 4608:5120 (rows<32) | wf1T 5120:9216 | wf2T 9216:11264 |
    # wo1T 11264:13312 | wo2T 13312:15360
    import ml_dtypes
    encpack0 = np.zeros((128, 15360), dtype=ml_dtypes.bfloat16)
    encpack0[:, 4096:4608] = _bf16(sh.pop("wsT"))
    encpack0[0:AD, 4608:5120] = _bf16(sh.pop("waT"))
    encpack0[:, 5120:9216] = _bf16(sh.pop("wf1T"))
    encpack0[:, 9216:11264] = _bf16(sh.pop("wf2T"))
    encpack0[:, 11264:13312] = _bf16(sh.pop("wo1T"))
    encpack0[:, 13312:15360] = _bf16(sh.pop("wo2T"))

    # --- per-core data ---
    state = _f32(state)
    action = _f32(action)
    x_pad = _f32(x_pad)
    cores = []
    for c in range(NCORES):
        s = state[c * BC:(c + 1) * BC]
        a = action[c * BC:(c + 1) * BC]
        x = x_pad[c * BC:(c + 1) * BC]
        d = dict(sh)
        ep = encpack0.copy()
        ep[:, 0:2048] = _bf16(s.T)
        ep[0:AD, 2048:4096] = _bf16(a.T)
        d["encpack"] = ep
        # batch-major [128, c*8+f]
        d["xpbm"] = _f32(x.reshape(NCH, 128, FD).transpose(1, 0, 2).reshape(128, NCH * FD))
        cores.append(d)

    host_const = ld_host - 0.5 * FD * float(np.log(2.0 * np.pi))
    return cores, host_const


# ---------------------------------------------------------------------------
# device program
# ---------------------------------------------------------------------------

def fr(ap):
    return ap


def build_nc():
    nc = bacc.Bacc("TRN2", target_bir_lowering=False, debug=False)
    dram = {}

    BF_NAMES = {"encpack", "a_sb", "w0a", "bfr", "flw"}

    def din(name, shape):
        dt = BF if name in BF_NAMES else F32
        dram[name] = nc.declare_dram_parameter(name, list(shape), dt, isOutput=False)
        return dram[name]

    din("encpack", (128, 15360))
    din("xpbm", (128, NCH * FD))
    din("encbp", (128, 24))
    din("encg", (128, 20))
    din("encbg", (128, 20))
    din("a_sb", (FD, L * FD))
    din("lub", (FD, L))
    din("w0a", (FD, L * H))
    din("fbp", (128, L * 14))
    din("bfr", (1, L * 256))
    din("flw", (L, 128, 5632))
    din("kbias", (128, K))
    din("kiota", (128, K))
    out_t = nc.declare_dram_parameter("logq_out", [128, NCH], F32, isOutput=True)

    with tile.TileContext(nc) as tc:
        _body(nc, tc, dram, out_t)
    nc.compile()
    return nc


def _body(nc, tc, dram, out_t):
    from contextlib import ExitStack
    ctx = ExitStack()
    with ctx:
        const = ctx.enter_context(tc.tile_pool(name="const", bufs=1))
        persist = ctx.enter_context(tc.tile_pool(name="persist", bufs=1))
        psum = ctx.enter_context(tc.tile_pool(name="psum", bufs=2, space="PSUM"))

        # ---- constants ----
        ident = const.tile([128, 128], F32, tag="ident", name="ident")
        make_identity(nc, ident)
        ones_col = const.tile([128, 1], BF, tag="ones_col", name="ones_col")
        nc.vector.memset(ones_col, 1.0)
        ones_row = const.tile([1, 512], F32, tag="ones_row", name="ones_row")
        nc.vector.memset(ones_row, 1.0)
        ones_rbf = const.tile([1, 128], BF, tag="ones_rbf", name="ones_rbf")
        nc.vector.memset(ones_rbf, 1.0)
        ident_bf = const.tile([128, 128], BF, tag="ident_bf", name="ident_bf")
        make_identity(nc, ident_bf)
        zeros128 = const.tile([128, 128], F32, tag="zeros128", name="zeros128")
        nc.vector.memset(zeros128, 0.0)
        kbias = const.tile([128, K], F32, tag="kbias", name="kbias")
        nc.sync.dma_start(out=kbias, in_=dram["kbias"][:, :])
        kiota = const.tile([128, K], F32, tag="kiota", name="kiota")
        nc.sync.dma_start(out=kiota, in_=dram["kiota"][:, :])
        epsc = const.tile([128, 1], F32, tag="epsc", name="epsc")
        nc.vector.memset(epsc, LN_EPS)

        # ---- persistent weights/data ----
        def load(name, shape, dt=F32):
            t = persist.tile(list(shape), dt, tag=name, name=name)
            nc.sync.dma_start(out=t, in_=dram[name][:, :])
            return t

        encbp = load("encbp", (128, 24))
        encg = load("encg", (128, 20))
        encbg = load("encbg", (128, 20))
        a_sb = load("a_sb", (FD, L * FD), BF)
        lub = load("lub", (FD, L))
        w0a = load("w0a", (FD, L * H), BF)
        fbp = load("fbp", (128, L * 14))
        bfr = load("bfr", (1, L * 256), BF)

        ctxT = persist.tile([128, 4, BC], BF, tag="ctxT", name="ctxT")       # encoder out

        # =========================== ENCODER ===========================
        with tc.tile_pool(name="encw", bufs=1) as encw, \
             tc.tile_pool(name="ework", bufs=2) as ework:

            encpack = encw.tile([128, 15360], BF, tag="encpack", name="encpack")
            nc.sync.dma_start(out=encpack, in_=dram["encpack"][:, :])
            stateT = encpack[:, 0:2048]
            actionT = encpack[0:AD, 2048:4096]
            wsT = encpack[:, 4096:4608]
            waT = encpack[0:AD, 4608:5120]
            wf1T = encpack[:, 5120:9216].rearrange("p (k n) -> p k n", n=512)
            wf2T = encpack[:, 9216:11264].rearrange("p (k n) -> p k n", n=512)
            wo1T = encpack[:, 11264:13312].rearrange("p (k n) -> p k n", n=512)
            wo2T = encpack[:, 13312:15360].rearrange("p (k n) -> p k n", n=512)

            encgv = encg[:, :].rearrange("p (l m) -> p l m", m=4)
            encbgv = encbg[:, :].rearrange("p (l m) -> p l m", m=4)
            encbpv = encbp[:, :].rearrange("p (l m) -> p l m", m=4)

            def mm_layer(rhs_fn, nkt, w_ap_fn, out_psums):
                """4 accumulating matmuls per out-chunk m."""
                for m in range(4):
                    pt = out_psums[m]
                    for kt in range(nkt):
                        nc.tensor.matmul(pt, fr(w_ap_fn(kt, m)), fr(rhs_fn(kt)),
                                         start=(kt == 0), stop=(kt == nkt - 1))

            def ln_relu(q, yps, lidx, bslot, out_tile):
                """LayerNorm+ReLU, feature-major, block q. yps: 4 psum [128,512]."""
                ysb = ework.tile([128, 4, 512], BF, tag="ysb", name="ysb")
                sq = ework.tile([128, 4, 512], BF, tag="sq", name="sq", bufs=1)
                for m in range(4):
                    nc.scalar.activation(ysb[:, m], yps[m], AF.Identity,
                                         bias=encbpv[:, bslot, m:m + 1])
                    nc.scalar.activation(sq[:, m], ysb[:, m], AF.Square)
                sps = psum.tile([1, 512], F32, tag="srow_ps", name="srow_ps", bufs=2)
                s2ps = psum.tile([1, 512], F32, tag="srow_ps", name="srow_ps", bufs=2)
                for m in range(4):
                    nc.tensor.matmul(sps, fr(ones_col[:, 0:1]), fr(ysb[:, m]),
                                     start=(m == 0), stop=(m == 3))
                for m in range(4):
                    nc.tensor.matmul(s2ps, fr(ones_col[:, 0:1]), fr(sq[:, m]),
                                     start=(m == 0), stop=(m == 3))
                srow = ework.tile([1, 512], F32, tag="srow", name="srow", bufs=1)
                srow2 = ework.tile([1, 512], F32, tag="srow2", name="srow2", bufs=1)
                nc.scalar.copy(srow, sps)
                nc.scalar.copy(srow2, s2ps)
                musq = ework.tile([1, 512], F32, tag="musq", name="musq", bufs=1)
                nc.scalar.activation(musq, srow, AF.Square, scale=1.0 / 512.0)
                varr = ework.tile([1, 512], F32, tag="varr", name="varr", bufs=1)
                nc.vector.scalar_tensor_tensor(varr, srow2, 1.0 / 512.0, musq,
                                               ALU.mult, ALU.subtract)
                sd = ework.tile([1, 512], F32, tag="sd", name="sd", bufs=1)
                nc.scalar.activation(sd, varr, AF.Sqrt, bias=epsc[0:1, 0:1])
                rstd = ework.tile([1, 512], F32, tag="rstd", name="rstd", bufs=1)
                nc.vector.reciprocal(rstd, sd)
                nmrs = ework.tile([1, 512], F32, tag="nmrs", name="nmrs", bufs=1)
                nc.vector.scalar_tensor_tensor(nmrs, srow, -1.0 / 512.0, rstd,
                                               ALU.mult, ALU.mult)
                rstdbc = psum.tile([128, 512], F32, tag="bcast", name="bcast", bufs=2)
                nc.tensor.matmul(rstdbc, fr(ones_row[0:1, 0:128]), fr(rstd),
                                 start=True, stop=True)
                nmrsbc = psum.tile([128, 512], F32, tag="bcast", name="bcast", bufs=2)
                nc.tensor.matmul(nmrsbc, fr(ones_row[0:1, 0:128]), fr(nmrs),
                                 start=True, stop=True)
                for m in range(4):
                    g_ap = encgv[:, lidx, m:m + 1]
                    v = ework.tile([128, 512], F32, tag="v", name="v")
                    w = ework.tile([128, 512], F32, tag="w", name="w")
                    nc.vector.scalar_tensor_tensor(v, ysb[:, m], g_ap, rstdbc,
                                                   ALU.mult, ALU.mult)
                    nc.vector.scalar_tensor_tensor(w, nmrsbc, g_ap, v,
                                                   ALU.mult, ALU.add)
                    nc.scalar.activation(out_tile[:, m], w, AF.Relu,
                                         bias=encbgv[:, lidx, m:m + 1])

            for q in range(NBLK):
                bsl = bass.ds(q * BLK, BLK)
                # s-path
                yps = [psum.tile([128, 512], F32, tag="ps512", name="ps512", bufs=2) for _ in range(4)]
                mm_layer(lambda kt: stateT[:, bsl], 1,
                         lambda kt, m: wsT[:, bass.ts(m, 128)], yps)
                s1 = ework.tile([128, 4, 512], BF, tag="hact", name="s1", bufs=4)
                ln_relu(q, yps, 0, 0, s1)
                # a-path
                yps = [psum.tile([128, 512], F32, tag="ps512", name="ps512", bufs=2) for _ in range(4)]
                mm_layer(lambda kt: actionT[:, bsl], 1,
                         lambda kt, m: waT[:, bass.ts(m, 128)], yps)
                a1 = ework.tile([128, 4, 512], BF, tag="hact", name="a1", bufs=4)
                ln_relu(q, yps, 1, 1, a1)
                # f1 (concat)
                yps = [psum.tile([128, 512], F32, tag="ps512", name="ps512", bufs=2) for _ in range(4)]
                mm_layer(lambda kt: s1[:, kt] if kt < 4 else a1[:, kt - 4], 8,
                         lambda kt, m: wf1T[:, kt, bass.ts(m, 128)], yps)
                h2 = ework.tile([128, 4, 512], BF, tag="hact", name="h2", bufs=4)
                ln_relu(q, yps, 2, 2, h2)
                # f2
                yps = [psum.tile([128, 512], F32, tag="ps512", name="ps512", bufs=2) for _ in range(4)]
                mm_layer(lambda kt: h2[:, kt], 4,
                         lambda kt, m: wf2T[:, kt, bass.ts(m, 128)], yps)
                h3 = ework.tile([128, 4, 512], BF, tag="hact", name="h3", bufs=4)
                ln_relu(q, yps, 3, 3, h3)
                # o1
                yps = [psum.tile([128, 512], F32, tag="ps512", name="ps512", bufs=2) for _ in range(4)]
                mm_layer(lambda kt: h3[:, kt], 4,
                         lambda kt, m: wo1T[:, kt, bass.ts(m, 128)], yps)
                h4 = ework.tile([128, 4, 512], BF, tag="hact", name="h4", bufs=4)
                ln_relu(q, yps, 4, 4, h4)
                # o2 (no LN)
                yps = [psum.tile([128, 512], F32, tag="ps512", name="ps512", bufs=2) for _ in range(4)]
                mm_layer(lambda kt: h4[:, kt], 4,
                         lambda kt, m: wo2T[:, kt, bass.ts(m, 128)], yps)
                for m in range(4):
                    nc.scalar.activation(ctxT[:, m, bsl], yps[m], AF.Identity,
                                         bias=encbpv[:, 5, m:m + 1])

        # =========================== FLOWS ===========================
        with tc.tile_pool(name="flw", bufs=2) as flwp, \
             tc.tile_pool(name="spl", bufs=1) as spl, \
             tc.tile_pool(name="fwork", bufs=2) as fwork:

            fbpv = fbp[:, :].rearrange("p (l s) -> p l s", s=14)

            zbm = spl.tile([128, NCH * FD], F32, tag="zbm", name="zbm")
            nc.sync.dma_start(out=zbm, in_=dram["xpbm"][:, :])
            ladacc = spl.tile([128, NCH * FD], F32, tag="ladacc", name="ladacc")
            nc.vector.memset(ladacc, 0.0)

            E = spl.tile([128, NCH, 256], F32, tag="E", name="E")
            CWP = spl.tile([128, NCH, 16, 9], F32, tag="CWP", name="CWP")
            nc.vector.memset(CWP[:, :, :, 0:1], -TB)
            D = spl.tile([128, NCH, 128], F32, tag="D", name="D")

            for i in range(L - 1, -1, -1):
                flw = flwp.tile([128, 5632], BF, tag="flw", name="flw")
                nc.sync.dma_start(out=flw, in_=dram["flw"][i])
                wc0 = flw[:, 0:1024].rearrange("p (k n) -> p k n", n=256)
                wb1 = flw[:, 1024:2048].rearrange("p (j k n) -> p j k n", j=2, n=256)
                wb2 = flw[:, 2048:3072].rearrange("p (j k n) -> p j k n", j=2, n=256)
                wcb = flw[:, 3072:5120].rearrange("p (j k n) -> p j k n", j=2, n=256)
                wft = flw[:, 5120:5632].rearrange("p (k n) -> p k n", n=256)

                # ---- z -> zT (PE transposes) ----
                zT = fwork.tile([FD, BC], BF, tag="zT", name="zT", bufs=1)
                for c in range(NCH):
                    pt = psum.tile([FD, 128], F32, tag="tsp", name="tsp", bufs=1)
                    nc.tensor.transpose(pt, zbm[:, c * FD:(c + 1) * FD], ident)
                    nc.scalar.copy(zT[:, c * 128:(c + 1) * 128], pt)
                # ---- LU: zpT = A^T z^T + b ----
                zpT = fwork.tile([FD, BC], BF, tag="zpT", name="zpT", bufs=1)
                for q in range(NBLK):
                    pt = psum.tile([FD, BLK], F32, tag="tsp", name="tsp", bufs=1)
                    nc.tensor.matmul(pt, fr(a_sb[:, i * FD:(i + 1) * FD]),
                                     fr(zT[:, bass.ds(q * BLK, BLK)]),
                                     start=True, stop=True)
                    nc.scalar.activation(zpT[:, bass.ds(q * BLK, BLK)], pt,
                                         AF.Identity, bias=lub[:, i:i + 1])
                # ---- z' batch-major (x for spline) ----
                xbm = spl.tile([128, NCH * FD], F32, tag="xbm", name="xbm")
                for c in range(NCH):
                    pt = psum.tile([128, FD], BF, tag="tsp", name="tsp", bufs=1)
                    nc.tensor.transpose(pt, zpT[:, c * 128:(c + 1) * 128],
                                        ident_bf[0:FD, 0:FD])
                    nc.scalar.copy(xbm[:, c * FD:(c + 1) * FD], pt)

                # ---- MADE per block ----
                for q in range(NBLK):
                    bsl = bass.ds(q * BLK, BLK)
                    hps = []
                    for m in range(2):
                        pt = psum.tile([128, 512], F32, tag="ps512", name="ps512", bufs=2)
                        nc.tensor.matmul(pt, fr(w0a[:, i * H + 128 * m: i * H + 128 * (m + 1)]),
                                         fr(zpT[:, bsl]), start=True, stop=False)
                        for kt in range(4):
                            nc.tensor.matmul(pt, fr(wc0[:, kt, bass.ts(m, 128)]),
                                             fr(ctxT[:, kt, bsl]),
                                             start=False, stop=(kt == 3))
                        hps.append(pt)
                    hsb = fwork.tile([128, 2, 512], BF, tag="hsb", name="hsb")
                    t1 = fwork.tile([128, 2, 512], BF, tag="t1", name="t1", bufs=1)
                    for m in range(2):
                        nc.scalar.activation(hsb[:, m], hps[m], AF.Identity,
                                             bias=fbpv[:, i, m:m + 1])
                        nc.scalar.activation(t1[:, m], hps[m], AF.Relu,
                                             bias=fbpv[:, i, m:m + 1])
                    for j in range(NB):
                        t2r = fwork.tile([128, 2, 512], BF, tag="t2r", name="t2r", bufs=1)
                        for m in range(2):
                            pt = psum.tile([128, 512], F32, tag="ps512", name="ps512", bufs=2)
                            for kt in range(2):
                                nc.tensor.matmul(pt, fr(wb1[:, j, kt, bass.ts(m, 128)]),
                                                 fr(t1[:, kt]),
                                                 start=(kt == 0), stop=(kt == 1))
                            nc.scalar.activation(t2r[:, m], pt, AF.Relu,
                                                 bias=fbpv[:, i, 2 + 6 * j + m:3 + 6 * j + m])
                        for m in range(2):
                            scg = fwork.tile([128, 512], F32, tag="scg", name="scg")
                            mt = fwork.tile([128, 512], BF, tag="mt", name="mt")
                            cp = psum.tile([128, 512], F32, tag="ps512", name="ps512", bufs=2)
                            for kt in range(4):
                                nc.tensor.matmul(cp, fr(wcb[:, j, kt, bass.ts(m, 128)]),
                                                 fr(ctxT[:, kt, bsl]),
                                                 start=(kt == 0), stop=(kt == 3))
                            nc.scalar.activation(scg, cp, AF.Sigmoid,
                                                 bias=fbpv[:, i, 6 + 6 * j + m:7 + 6 * j + m])
                            t3p = psum.tile([128, 512], F32, tag="ps512", name="ps512", bufs=2)
                            for kt in range(2):
                                nc.tensor.matmul(t3p, fr(wb2[:, j, kt, bass.ts(m, 128)]),
                                                 fr(t2r[:, kt]),
                                                 start=(kt == 0), stop=(kt == 1))
                            nc.vector.scalar_tensor_tensor(
                                mt, t3p, fbpv[:, i, 4 + 6 * j + m:5 + 6 * j + m],
                                scg, ALU.add, ALU.mult)
                            nc.gpsimd.tensor_add(hsb[:, m], hsb[:, m], mt)
                            if j == 0:
                                nc.gpsimd.tensor_relu(t1[:, m], hsb[:, m])
                    # params -> E (batch-major, via Exp)
                    for c4 in range(4):
                        cg = q * 4 + c4
                        pp = psum.tile([128, 256], F32, tag="pmm", name="pmm", bufs=1)
                        for kt in range(2):
                            nc.tensor.matmul(pp, fr(hsb[:, kt, bass.ts(c4, 128)]),
                                             fr(wft[:, kt, :]),
                                             start=(kt == 0), stop=False)
                        nc.tensor.matmul(pp, fr(ones_rbf[0:1, :]),
                                         fr(bfr[0:1, i * 256:(i + 1) * 256]),
                                         start=False, stop=True)
                        nc.scalar.activation(E[:, cg, :], pp, AF.Exp)

                # ---- spline (whole core) ----
                Ewh = E[:, :, 0:128].rearrange("p c (g k) -> p c g k", k=K)
                for k in range(1, K):
                    nc.vector.tensor_tensor(Ewh[:, :, :, k], Ewh[:, :, :, k],
                                            Ewh[:, :, :, k - 1], ALU.add)
                Rt = spl.tile([128, NCH, 16], F32, tag="Rt", name="Rt")
                nc.vector.reciprocal(Rt, Ewh[:, :, :, K - 1])
                nc.vector.scalar_tensor_tensor(
                    CWP[:, :, :, 1:9], Ewh, CW_SCALE,
                    Rt[:, :, :].unsqueeze(3).broadcast_to([128, NCH, 16, K]),
                    ALU.mult, ALU.mult)
                nc.vector.tensor_tensor(
                    CWP[:, :, :, 1:9], CWP[:, :, :, 1:9],
                    kbias[:, :].unsqueeze(1).unsqueeze(1).broadcast_to([128, NCH, 16, K]),
                    ALU.add)
                # derivatives: D = MD + ln(1 + E_d)
                nc.scalar.activation(D[:, :, :], E[:, :, 128:256], AF.Ln, bias=1.0)
                nc.vector.tensor_scalar_add(D[:, :, :], D[:, :, :], MD)

                xv = xbm[:, :].rearrange("p (c f) -> p c f", f=FD)
                XC = spl.tile([128, NCH, FD], F32, tag="XC", name="XC")
                nc.vector.tensor_scalar(XC, xv, TB, -TB, ALU.min, ALU.max)
                CMPT = spl.tile([128, NCH, FD, 7], F32, tag="gprod", name="CMPT")
                nc.vector.tensor_tensor(
                    CMPT, XC[:, :, :].unsqueeze(3).broadcast_to([128, NCH, FD, 7]),
                    CWP[:, :, 0:8, 1:8], ALU.is_ge)
                IDX = spl.tile([128, NCH, FD], F32, tag="IDX", name="IDX")
                nc.vector.tensor_reduce(IDX, CMPT, mybir.AxisListType.X, ALU.add)
                OH = spl.tile([128, NCH, FD, K], F32, tag="OH", name="OH")
                nc.vector.tensor_tensor(
                    OH, IDX[:, :, :].unsqueeze(3).broadcast_to([128, NCH, FD, K]),
                    kiota[:, :].unsqueeze(1).unsqueeze(1).broadcast_to([128, NCH, FD, K]),
                    ALU.is_equal)

                def gather(src, out, use_gps):
                    prod = spl.tile([128, NCH, FD, K], F32, tag="gprod", name="gprod")
                    eng = nc.gpsimd if use_gps else nc.vector
                    eng.tensor_tensor(prod, OH, src, ALU.mult)
                    nc.vector.tensor_reduce(out, prod, mybir.AxisListType.X, ALU.add)
                    return out

                def stile(tag):
                    return spl.tile([128, NCH, FD], F32, tag=tag, name=tag)

                icw = gather(CWP[:, :, 0:8, 0:8], stile("icw"), False)
                icwR = gather(CWP[:, :, 0:8, 1:9], stile("icwR"), True)
                ich = gather(CWP[:, :, 8:16, 0:8], stile("ich"), False)
                ichR = gather(CWP[:, :, 8:16, 1:9], stile("ichR"), True)
                Dv = D[:, :, :].rearrange("p c (h x) -> p c h x", h=2)
                dk = gather(Dv[:, :, 0].rearrange("p c (f k) -> p c f k", k=K),
                            stile("dk"), False)
                dk1 = gather(Dv[:, :, 1].rearrange("p c (f k) -> p c f k", k=K),
                             stile("dk1"), True)

                TT = nc.vector.tensor_tensor
                STT = nc.vector.scalar_tensor_tensor
                iw = stile("iw"); TT(iw, icwR, icw, ALU.subtract)
                ih = stile("ih"); TT(ih, ichR, ich, ALU.subtract)
                riw = stile("riw"); nc.vector.reciprocal(riw, iw)
                delta = stile("delta"); TT(delta, ih, riw, ALU.mult)
                tmp = stile("tmp"); TT(tmp, XC, icw, ALU.subtract)
                th = stile("th"); TT(th, tmp, riw, ALU.mult)
                u = stile("u"); TT(u, th, th, ALU.mult)
                th1 = stile("th1"); TT(th1, th, u, ALU.subtract)
                s = stile("s"); TT(s, dk, dk1, ALU.add)
                s2 = stile("s2"); STT(s2, delta, -2.0, s, ALU.mult, ALU.add)
                dn1 = stile("dn1"); TT(dn1, s2, th1, ALU.mult)
                den = stile("den"); TT(den, dn1, delta, ALU.add)
                aa = stile("aa"); TT(aa, delta, u, ALU.mult)
                bb = stile("bb"); TT(bb, dk, th1, ALU.mult)
                num = stile("num"); TT(num, aa, bb, ALU.add)
                rden = stile("rden"); nc.vector.reciprocal(rden, den)
                fr_ = stile("fr"); TT(fr_, num, rden, ALU.mult)
                t5 = stile("t5"); TT(t5, ih, fr_, ALU.mult)
                outs = spl.tile([128, NCH * FD], F32, tag=f"outs{i % 2}", name=f"outs{i % 2}")
                outsv = outs[:, :].rearrange("p (c f) -> p c f", f=FD)
                TT(outsv, ich, t5, ALU.add)
                vv = stile("vv"); STT(vv, th, -2.0, u, ALU.mult, ALU.add)
                g1 = stile("g1"); TT(g1, dk1, u, ALU.mult)
                g2 = stile("g2"); STT(g2, delta, 2.0, th1, ALU.mult, ALU.mult)
                g3 = stile("g3"); TT(g3, dk, vv, ALU.mult)
                i1 = stile("i1"); TT(i1, g1, g2, ALU.add)
                i2 = stile("i2"); TT(i2, i1, g3, ALU.add)
                inner = stile("inner"); TT(inner, i2, dk, ALU.add)
                d2 = stile("d2"); TT(d2, delta, delta, ALU.mult)
                dnum = stile("dnum"); TT(dnum, d2, inner, ALU.mult)
                l1 = stile("l1")
                nc.scalar.activation(l1, dnum, AF.Ln)
                l2 = stile("l2")
                nc.scalar.activation(l2, den, AF.Ln)
                lad = stile("lad"); STT(lad, l2, -2.0, l1, ALU.mult, ALU.add)
                absx = spl.tile([128, NCH * FD], F32, tag="absx", name="absx")
                nc.scalar.activation(absx, xbm, AF.Abs)
                msk = spl.tile([128, NCH * FD], mybir.dt.uint8, tag="msk", name="msk")
                nc.vector.tensor_scalar(msk, absx, TB, None, ALU.is_gt)
                mskv = msk[:, :].rearrange("p (c f) -> p c f", f=FD)
                nc.vector.copy_predicated(outsv, mskv,
                                          xbm[:, :].rearrange("p (c f) -> p c f", f=FD))
                nc.vector.copy_predicated(lad, mskv,
                                          zeros128[:, :].rearrange("p (c f) -> p c f", f=FD))
                lav = ladacc[:, :].rearrange("p (c f) -> p c f", f=FD)
                nc.vector.tensor_tensor(lav, lav, lad, ALU.add)
                zbm = outs  # next layer's z

            # ---- final logq ----
            zsq = spl.tile([128, NCH, FD], F32, tag="zsq", name="zsq")
            zv = zbm[:, :].rearrange("p (c f) -> p c f", f=FD)
            nc.vector.tensor_tensor(zsq, zv, zv, ALU.mult)
            zr = spl.tile([128, NCH], F32, tag="zr", name="zr")
            nc.vector.tensor_reduce(zr, zsq, mybir.AxisListType.X, ALU.add)
            lr = spl.tile([128, NCH], F32, tag="lr", name="lr")
            nc.vector.tensor_reduce(lr, ladacc[:, :].rearrange("p (c f) -> p c f", f=FD),
                                    mybir.AxisListType.X, ALU.add)
            logq = spl.tile([128, NCH], F32, tag="logq", name="logq")
            nc.vector.scalar_tensor_tensor(logq, zr, -0.5, lr, ALU.mult, ALU.add)
            nc.sync.dma_start(out=out_t[:, :], in_=logq)


_NC_CACHE = None


def _get_nc():
    global _NC_CACHE
    if _NC_CACHE is None:
        _NC_CACHE = build_nc()
    return _NC_CACHE


def kernel_logq(**inputs):
    """Full per-sample logq (device part only) — for debugging/assembly."""
    cores, host_const = prep_host(**inputs)
    nc = _get_nc()
    res = run_bass_kernel_spmd(nc, cores, list(range(NCORES)))
    parts = []
    for c in range(NCORES):
        lq = res.results[c]["logq_out"]          # [128, 16] = [p, chunk]
        parts.append(np.asarray(lq).T.reshape(BC))  # chunk-major rows
    return np.concatenate(parts), host_const


def kernel(**inputs):
    logq, host_const = kernel_logq(**inputs)
    total = float(np.mean(logq.astype(np.float64))) + host_const
    return np.float32(-total)


# revision 22
# speedup vs baseline: 1.4066x; 1.4066x over previous
"""Trainium2 Bass kernel for nn_ConditionalNSF (conditional neural spline flow NLL).

Strategy: pure data parallel over 8 NeuronCores (2048 rows each).
 - Host: tiny param prep (MADE masks, LU-fold into MADE layer 0, final-layer
   column reorder + spline boundary-derivative bias trick), final scalar mean.
 - Device, per core:
   * feature-major encoder MLP (activations [feat, batch]) with float32r
     matmuls at N=512; LayerNorm stats via PE ones-matmuls + ACT Square.
   * 4 flow layers: LU step folded into an 8x8 matmul; MADE residual MLP
     feature-major; final MADE matmul emits batch-major spline params
     (swapped operands, N=256) straight into an Exp activation.
   * rational-quadratic spline fully batch-major: cumsum via strided adds,
     bin search + one-hot gathers on DVE/GPSIMD, exp/ln/softplus on ACT.
   * per-row logq accumulated on chip, [128,16] DMA'd out per core.
"""
import sys

sys.path.insert(0, "/opt/trn_rl_repo")

import numpy as np

import concourse.bass as bass
import concourse.bacc as bacc
import concourse.tile as tile
from concourse import mybir
from concourse.bass_utils import run_bass_kernel_spmd
from concourse.masks import make_identity

AF = mybir.ActivationFunctionType
ALU = mybir.AluOpType
F32 = mybir.dt.float32
BF = mybir.dt.bfloat16

# ---- problem dims (hardcoded) ----
B = 16384
NCORES = 8
BC = B // NCORES          # 2048 rows per core
NBLK = 4                  # batch blocks of 512 per core
BLK = 512
NCH = 16                  # chunks of 128 per core
SD, AD, CTX, HID = 128, 32, 512, 512
FD, H, NB, K, L = 8, 256, 2, 8, 4
M = 3 * K - 1
TB = 6.0
MBW = MBH = MD = 1e-3
LN_EPS = 1e-5
SQH = float(np.sqrt(H))
CPAD = float(np.log(np.expm1(1.0 - MD)))
CW_SCALE = 2.0 * TB * (1.0 - MBW * K)   # cum * rec * this + kbias


def _masks():
    in_deg = np.arange(1, FD + 1)
    hid_deg = np.arange(H) % max(1, FD - 1) + min(1, FD - 1)
    m0 = (hid_deg[:, None] >= in_deg[None, :]).astype(np.float32)
    mh = (hid_deg[:, None] >= hid_deg[None, :]).astype(np.float32)
    out_deg = np.repeat(in_deg, M)
    mf = (out_deg[:, None] > hid_deg[None, :]).astype(np.float32)
    return m0, mh, mf


def _softplus(x):
    return np.log1p(np.exp(x))


def _f32(x):
    return np.ascontiguousarray(np.asarray(x, np.float32))


def _bf16(x):
    import ml_dtypes
    return np.ascontiguousarray(np.asarray(x, np.float32).astype(ml_dtypes.bfloat16))


def prep_host(state, action, x_pad, enc_params, flow_params, perms):
    """Returns (shared_inputs dict, per_core list of dicts, host_const float)."""
    p = {k: _f32(v) for k, v in enc_params.items()}
    fp = {k: _f32(v) for k, v in flow_params.items()}
    perms = np.asarray(perms)
    m0, mh, mf = _masks()

    sh = {}
    # --- encoder weights, transposed [D_in, D_out] ---
    sh["wsT"] = _f32(p["Ws"].T)                               # [128, 512]
    sh["waT"] = _f32(p["Wa"].T)                               # [32, 512]
    sh["wf1T"] = _f32(p["Wf1"].T.reshape(8, 128, 512).transpose(1, 0, 2)
                      .reshape(128, 8 * 512))                 # [128, kt*512]
    for nm, tag in (("Wf2", "wf2T"), ("Wo1", "wo1T"), ("Wo2", "wo2T")):
        sh[tag] = _f32(p[nm].T.reshape(4, 128, 512).transpose(1, 0, 2)
                       .reshape(128, 4 * 512))
    # encoder biases [128, 6, 4] (slot l, chunk m)
    encbp = np.zeros((128, 6, 4), np.float32)
    for l, nm in enumerate(["bs", "ba", "bf1", "bf2", "bo1", "bo2"]):
        encbp[:, l, :] = p[nm].reshape(4, 128).T
    sh["encbp"] = _f32(encbp.reshape(128, 24))
    encg = np.zeros((128, 5, 4), np.float32)
    encbg = np.zeros((128, 5, 4), np.float32)
    for l, (g, bg) in enumerate([("gs", "bgs"), ("ga", "bga"), ("gf1", "bgf1"),
                                 ("gf2", "bgf2"), ("go1", "bgo1")]):
        encg[:, l, :] = p[g].reshape(4, 128).T
        encbg[:, l, :] = p[bg].reshape(4, 128).T
    sh["encg"] = _f32(encg.reshape(128, 20))
    sh["encbg"] = _f32(encbg.reshape(128, 20))

    # --- flow prep ---
    A = np.zeros((L, FD, FD), np.float32)
    ld_host = 0.0
    for i in range(L):
        Pm = np.zeros((FD, FD), np.float32)
        Pm[perms[i], np.arange(FD)] = 1.0
        lw = np.tril(fp["lu_L"][i], -1) + np.eye(FD, dtype=np.float32)
        udiag = _softplus(fp["lu_d"][i]) + 1e-3
        up = np.triu(fp["lu_U"][i], 1) + np.diag(udiag)
        A[i] = Pm @ up.T @ lw.T
        ld_host += float(np.sum(np.log(udiag)))
    sh["a_sb"] = _f32(A.transpose(1, 0, 2).reshape(FD, L * FD))   # [8, L*8]
    sh["lub"] = _f32(fp["lu_b"].T)                                # [8, L]

    W0m = fp["W0"] * m0[None]
    w0a = np.stack([A[i] @ W0m[i].T for i in range(L)])           # [L, 8, H]
    sh["w0a"] = _f32(w0a.transpose(1, 0, 2).reshape(FD, L * H))   # [8, L*256]
    bias0 = np.stack([fp["lu_b"][i] @ W0m[i].T + fp["b0"][i] + fp["bc0"][i]
                      for i in range(L)])                         # [L, H]

    # per-partition bias pack [128, L, 14]:
    # slots: 0-1 bias0(m), then per j (j*6): 2+6j bb1(m), 4+6j bb2(m)(UNUSED,
    # bb2 folded into mt STT), ... keep simple: 0-1 bias0, 2-3 bb1 j0,
    # 4-5 bb2 j0, 6-7 bcb j0, 8-9 bb1 j1, 10-11 bb2 j1, 12-13 bcb j1
    fbp = np.zeros((128, L, 14), np.float32)
    for i in range(L):
        fbp[:, i, 0:2] = bias0[i].reshape(2, 128).T
        for j in range(NB):
            fbp[:, i, 2 + 6 * j:4 + 6 * j] = fp["bb1"][i, j].reshape(2, 128).T
            fbp[:, i, 4 + 6 * j:6 + 6 * j] = fp["bb2"][i, j].reshape(2, 128).T
            fbp[:, i, 6 + 6 * j:8 + 6 * j] = fp["bcb"][i, j].reshape(2, 128).T
    sh["fbp"] = _f32(fbp.reshape(128, L * 14))

    # final-layer rhs [H, 256] per layer: cols [UW(64)|UH(64)|D0(64)|D1(64)]
    Wfm = fp["Wf"] * mf[None]
    bf = fp["bf"]
    wft = np.zeros((L, H, 256), np.float32)
    bfr = np.zeros((L, 256), np.float32)
    for i in range(L):
        for f in range(FD):
            for k in range(K):
                wft[i, :, f * K + k] = Wfm[i, f * M + k] / SQH
                bfr[i, f * K + k] = bf[i, f * M + k] / SQH
                wft[i, :, 64 + f * K + k] = Wfm[i, f * M + K + k] / SQH
                bfr[i, 64 + f * K + k] = bf[i, f * M + K + k] / SQH
                if k == 0:
                    bfr[i, 128 + f * K] = CPAD
                else:
                    wft[i, :, 128 + f * K + k] = Wfm[i, f * M + 2 * K + k - 1]
                    bfr[i, 128 + f * K + k] = bf[i, f * M + 2 * K + k - 1]
                if k == K - 1:
                    bfr[i, 192 + f * K + k] = CPAD
                else:
                    wft[i, :, 192 + f * K + k] = Wfm[i, f * M + 2 * K + k]
                    bfr[i, 192 + f * K + k] = bf[i, f * M + 2 * K + k]
    sh["bfr"] = _f32(bfr.reshape(1, L * 256))

    # streamed per-layer weight pack [L, 128, 5632]:
    # cols: wc0 (kt4,256)=0:1024 | wb1 (j2,kt2,256)=1024:2048 |
    #       wb2 (j2,kt2,256)=2048:3072 | wcb (j2,kt4,256)=3072:5120 |
    #       wft (kt2,256)=5120:5632
    flw = np.zeros((L, 128, 5632), np.float32)
    for i in range(L):
        wc0T = fp["Wc0"][i].T            # [CTX, H]
        flw[i, :, 0:1024] = wc0T.reshape(4, 128, 256).transpose(1, 0, 2).reshape(128, 1024)
        for j in range(NB):
            b1 = (fp["Wb1"][i, j] * mh).T    # [H, H]
            b2 = (fp["Wb2"][i, j] * mh).T
            cb = fp["Wcb"][i, j].T           # [CTX, H]
            flw[i, :, 1024 + 512 * j:1024 + 512 * (j + 1)] = \
                b1.reshape(2, 128, 256).transpose(1, 0, 2).reshape(128, 512)
            flw[i, :, 2048 + 512 * j:2048 + 512 * (j + 1)] = \
                b2.reshape(2, 128, 256).transpose(1, 0, 2).reshape(128, 512)
            flw[i, :, 3072 + 1024 * j:3072 + 1024 * (j + 1)] = \
                cb.reshape(4, 128, 256).transpose(1, 0, 2).reshape(128, 1024)
        flw[i, :, 5120:5632] = wft[i].reshape(2, 128, 256).transpose(1, 0, 2).reshape(128, 512)
    sh["flw"] = _f32(flw)

    # spline consts
    kk = np.arange(1, K + 1, dtype=np.float32)
    sh["kbias"] = _f32(np.broadcast_to(2 * TB * MBW * kk - TB, (128, K)))
    sh["kiota"] = _f32(np.broadcast_to(np.arange(K, dtype=np.float32), (128, K)))

    for k in ["a_sb", "w0a", "bfr", "flw"]:
        sh[k] = _bf16(sh[k])

    # single packed bf16 tensor for ALL encoder-phase DMAs (one DMA proc ->
    # small released-zone dep sets). col layout:
    # state 0:2048 | action 2048:4096 (rows<32) | wsT 4096:4608 |
    # waT 4608:5120 (rows<32) | wf1T 5120:9216 | wf2T 9216:11264 |
    # wo1T 11264:13312 | wo2T 13312:15360
    import ml_dtypes
    encpack0 = np.zeros((128, 15360), dtype=ml_dtypes.bfloat16)
    encpack0[:, 4096:4608] = _bf16(sh.pop("wsT"))
    encpack0[0:AD, 4608:5120] = _bf16(sh.pop("waT"))
    encpack0[:, 5120:9216] = _bf16(sh.pop("wf1T"))
    encpack0[:, 9216:11264] = _bf16(sh.pop("wf2T"))
    encpack0[:, 11264:13312] = _bf16(sh.pop("wo1T"))
    encpack0[:, 13312:15360] = _bf16(sh.pop("wo2T"))

    # --- per-core data ---
    state = _f32(state)
    action = _f32(action)
    x_pad = _f32(x_pad)
    cores = []
    for c in range(NCORES):
        s = state[c * BC:(c + 1) * BC]
        a = action[c * BC:(c + 1) * BC]
        x = x_pad[c * BC:(c + 1) * BC]
        d = dict(sh)
        ep = encpack0.copy()
        ep[:, 0:2048] = _bf16(s.T)
        ep[0:AD, 2048:4096] = _bf16(a.T)
        d["encpack"] = ep
        # batch-major [128, c*8+f]
        d["xpbm"] = _f32(x.reshape(NCH, 128, FD).transpose(1, 0, 2).reshape(128, NCH * FD))
        cores.append(d)

    host_const = ld_host - 0.5 * FD * float(np.log(2.0 * np.pi))
    return cores, host_const


# ---------------------------------------------------------------------------
# device program
# ---------------------------------------------------------------------------

def fr(ap):
    return ap


def build_nc():
    nc = bacc.Bacc("TRN2", target_bir_lowering=False, debug=False)
    dram = {}

    BF_NAMES = {"encpack", "a_sb", "w0a", "bfr", "flw"}

    def din(name, shape):
        dt = BF if name in BF_NAMES else F32
        dram[name] = nc.declare_dram_parameter(name, list(shape), dt, isOutput=False)
        return dram[name]

    din("encpack", (128, 15360))
    din("xpbm", (128, NCH * FD))
    din("encbp", (128, 24))
    din("encg", (128, 20))
    din("encbg", (128, 20))
    din("a_sb", (FD, L * FD))
    din("lub", (FD, L))
    din("w0a", (FD, L * H))
    din("fbp", (128, L * 14))
    din("bfr", (1, L * 256))
    din("flw", (L, 128, 5632))
    din("kbias", (128, K))
    din("kiota", (128, K))
    out_t = nc.declare_dram_parameter("logq_out", [128, NCH], F32, isOutput=True)

    with tile.TileContext(nc) as tc:
        _body(nc, tc, dram, out_t)
    nc.compile()
    return nc


def _body(nc, tc, dram, out_t):
    from contextlib import ExitStack
    ctx = ExitStack()
    with ctx:
        const = ctx.enter_context(tc.tile_pool(name="const", bufs=1))
        persist = ctx.enter_context(tc.tile_pool(name="persist", bufs=1))
        psum = ctx.enter_context(tc.tile_pool(name="psum", bufs=2, space="PSUM"))

        # ---- constants ----
        ident = const.tile([128, 128], F32, tag="ident", name="ident")
        make_identity(nc, ident)
        ones_col = const.tile([128, 1], BF, tag="ones_col", name="ones_col")
        nc.vector.memset(ones_col, 1.0)
        ones_row = const.tile([1, 512], F32, tag="ones_row", name="ones_row")
        nc.vector.memset(ones_row, 1.0)
        ones_rbf = const.tile([1, 128], BF, tag="ones_rbf", name="ones_rbf")
        nc.vector.memset(ones_rbf, 1.0)
        ident_bf = const.tile([128, 128], BF, tag="ident_bf", name="ident_bf")
        make_identity(nc, ident_bf)
        zeros128 = const.tile([128, 128], F32, tag="zeros128", name="zeros128")
        nc.vector.memset(zeros128, 0.0)
        kbias = const.tile([128, K], F32, tag="kbias", name="kbias")
        nc.sync.dma_start(out=kbias, in_=dram["kbias"][:, :])
        kiota = const.tile([128, K], F32, tag="kiota", name="kiota")
        nc.sync.dma_start(out=kiota, in_=dram["kiota"][:, :])
        epsc = const.tile([128, 1], F32, tag="epsc", name="epsc")
        nc.vector.memset(epsc, LN_EPS)

        # ---- persistent weights/data ----
        def load(name, shape, dt=F32):
            t = persist.tile(list(shape), dt, tag=name, name=name)
            nc.sync.dma_start(out=t, in_=dram[name][:, :])
            return t

        encbp = load("encbp", (128, 24))
        encg = load("encg", (128, 20))
        encbg = load("encbg", (128, 20))
        a_sb = load("a_sb", (FD, L * FD), BF)
        lub = load("lub", (FD, L))
        w0a = load("w0a", (FD, L * H), BF)
        fbp = load("fbp", (128, L * 14))
        bfr = load("bfr", (1, L * 256), BF)

        ctxT = persist.tile([128, 4, BC], BF, tag="ctxT", name="ctxT")       # encoder out

        # =========================== ENCODER ===========================
        with tc.tile_pool(name="encw", bufs=1) as encw, \
             tc.tile_pool(name="ework", bufs=2) as ework:

            encpack = encw.tile([128, 15360], BF, tag="encpack", name="encpack")
            nc.sync.dma_start(out=encpack, in_=dram["encpack"][:, :])
            stateT = encpack[:, 0:2048]
            actionT = encpack[0:AD, 2048:4096]
            wsT = encpack[:, 4096:4608]
            waT = encpack[0:AD, 4608:5120]
            wf1T = encpack[:, 5120:9216].rearrange("p (k n) -> p k n", n=512)
            wf2T = encpack[:, 9216:11264].rearrange("p (k n) -> p k n", n=512)
            wo1T = encpack[:, 11264:13312].rearrange("p (k n) -> p k n", n=512)
            wo2T = encpack[:, 13312:15360].rearrange("p (k n) -> p k n", n=512)

            encgv = encg[:, :].rearrange("p (l m) -> p l m", m=4)
            encbgv = encbg[:, :].rearrange("p (l m) -> p l m", m=4)
            encbpv = encbp[:, :].rearrange("p (l m) -> p l m", m=4)

            def mm_layer(rhs_fn, nkt, w_ap_fn, out_psums):
                """4 accumulating matmuls per out-chunk m."""
                for m in range(4):
                    pt = out_psums[m]
                    for kt in range(nkt):
                        nc.tensor.matmul(pt, fr(w_ap_fn(kt, m)), fr(rhs_fn(kt)),
                                         start=(kt == 0), stop=(kt == nkt - 1))

            def ln_relu(q, yps, lidx, bslot, out_tile):
                """LayerNorm+ReLU, feature-major, block q. yps: 4 psum [128,512].

                rstd computed as exp(-0.5*ln(var+eps)) to stay in the
                natural_log_exp ACT table set and avoid DVE iterative divide.
                """
                ysb = ework.tile([128, 4, 512], BF, tag="ysb", name="ysb", bufs=3)
                sq = ework.tile([128, 4, 512], BF, tag="sq", name="sq", bufs=2)
                for m in range(4):
                    nc.scalar.activation(ysb[:, m], yps[m], AF.Identity,
                                         bias=encbpv[:, bslot, m:m + 1])
                    nc.vector.tensor_tensor(sq[:, m], ysb[:, m], ysb[:, m], ALU.mult)
                sps = psum.tile([33, 512], F32, tag="srow_ps", name="srow_ps", bufs=1)
                for m in range(4):
                    nc.tensor.matmul(sps[0:1, :], fr(ones_col[:, 0:1]), fr(ysb[:, m]),
                                     start=(m == 0), stop=(m == 3))
                for m in range(4):
                    nc.tensor.matmul(sps[32:33, :], fr(ones_col[:, 0:1]), fr(sq[:, m]),
                                     start=(m == 0), stop=(m == 3))
                srow = ework.tile([1, 512], F32, tag="srow", name="srow", bufs=2)
                nc.scalar.copy(srow, sps[0:1])
                srow2 = ework.tile([1, 512], F32, tag="srow2", name="srow2", bufs=2)
                nc.scalar.copy(srow2, sps[32:33])
                musq = ework.tile([1, 512], F32, tag="musq", name="musq", bufs=2)
                nc.scalar.activation(musq, srow, AF.Square, scale=1.0 / 512.0)
                varr = ework.tile([1, 512], F32, tag="varr", name="varr", bufs=2)
                nc.vector.scalar_tensor_tensor(varr, srow2, 1.0 / 512.0, musq,
                                               ALU.mult, ALU.subtract)
                lnv = ework.tile([1, 512], F32, tag="lnv", name="lnv", bufs=2)
                nc.scalar.activation(lnv, varr, AF.Ln, bias=epsc[0:1, 0:1])
                rstd = ework.tile([1, 512], F32, tag="rstd", name="rstd", bufs=2)
                nc.scalar.activation(rstd, lnv, AF.Exp, scale=-0.5)
                nmrs = ework.tile([1, 512], F32, tag="nmrs", name="nmrs", bufs=2)
                nc.vector.scalar_tensor_tensor(nmrs, srow, -1.0 / 512.0, rstd,
                                               ALU.mult, ALU.mult)
                rstdbc = psum.tile([128, 512], F32, tag="ps512", name="rstdbc", bufs=5)
                nc.tensor.matmul(rstdbc, fr(ones_row[0:1, 0:128]), fr(rstd),
                                 start=True, stop=True)
                nmrsbc = psum.tile([128, 512], F32, tag="ps512", name="nmrsbc", bufs=5)
                nc.tensor.matmul(nmrsbc, fr(ones_row[0:1, 0:128]), fr(nmrs),
                                 start=True, stop=True)
                for m in range(4):
                    g_ap = encgv[:, lidx, m:m + 1]
                    v = ework.tile([128, 512], F32, tag="v", name="v", bufs=3)
                    w = ework.tile([128, 512], F32, tag="w", name="w", bufs=3)
                    nc.vector.scalar_tensor_tensor(v, ysb[:, m], g_ap, rstdbc,
                                                   ALU.mult, ALU.mult)
                    nc.vector.scalar_tensor_tensor(w, nmrsbc, g_ap, v,
                                                   ALU.mult, ALU.add)
                    nc.scalar.activation(out_tile[:, m], w, AF.Relu,
                                         bias=encbgv[:, lidx, m:m + 1])

            for q in range(NBLK):
                bsl = bass.ds(q * BLK, BLK)
                # s-path
                yps = [psum.tile([128, 512], F32, tag="ps512", name="ps512", bufs=5) for _ in range(4)]
                mm_layer(lambda kt: stateT[:, bsl], 1,
                         lambda kt, m: wsT[:, bass.ts(m, 128)], yps)
                s1 = ework.tile([128, 4, 512], BF, tag="hact", name="s1", bufs=6)
                ln_relu(q, yps, 0, 0, s1)
                # a-path
                yps = [psum.tile([128, 512], F32, tag="ps512", name="ps512", bufs=5) for _ in range(4)]
                mm_layer(lambda kt: actionT[:, bsl], 1,
                         lambda kt, m: waT[:, bass.ts(m, 128)], yps)
                a1 = ework.tile([128, 4, 512], BF, tag="hact", name="a1", bufs=6)
                ln_relu(q, yps, 1, 1, a1)
                # f1 (concat)
                yps = [psum.tile([128, 512], F32, tag="ps512", name="ps512", bufs=5) for _ in range(4)]
                mm_layer(lambda kt: s1[:, kt] if kt < 4 else a1[:, kt - 4], 8,
                         lambda kt, m: wf1T[:, kt, bass.ts(m, 128)], yps)
                h2 = ework.tile([128, 4, 512], BF, tag="hact", name="h2", bufs=6)
                ln_relu(q, yps, 2, 2, h2)
                # f2
                yps = [psum.tile([128, 512], F32, tag="ps512", name="ps512", bufs=5) for _ in range(4)]
                mm_layer(lambda kt: h2[:, kt], 4,
                         lambda kt, m: wf2T[:, kt, bass.ts(m, 128)], yps)
                h3 = ework.tile([128, 4, 512], BF, tag="hact", name="h3", bufs=6)
                ln_relu(q, yps, 3, 3, h3)
                # o1
                yps = [psum.tile([128, 512], F32, tag="ps512", name="ps512", bufs=5) for _ in range(4)]
                mm_layer(lambda kt: h3[:, kt], 4,
                         lambda kt, m: wo1T[:, kt, bass.ts(m, 128)], yps)
                h4 = ework.tile([128, 4, 512], BF, tag="hact", name="h4", bufs=6)
                ln_relu(q, yps, 4, 4, h4)
                # o2 (no LN)
                yps = [psum.tile([128, 512], F32, tag="ps512", name="ps512", bufs=5) for _ in range(4)]
                mm_layer(lambda kt: h4[:, kt], 4,
                         lambda kt, m: wo2T[:, kt, bass.ts(m, 128)], yps)
                for m in range(4):
                    nc.scalar.activation(ctxT[:, m, bsl], yps[m], AF.Identity,
                                         bias=encbpv[:, 5, m:m + 1])

        # =========================== FLOWS ===========================
        with tc.tile_pool(name="flw", bufs=2) as flwp, \
             tc.tile_pool(name="spl", bufs=1) as spl, \
             tc.tile_pool(name="fwork", bufs=2) as fwork:

            fbpv = fbp[:, :].rearrange("p (l s) -> p l s", s=14)

            zbm = spl.tile([128, NCH * FD], F32, tag="zbm", name="zbm")
            nc.sync.dma_start(out=zbm, in_=dram["xpbm"][:, :])
            ladacc = spl.tile([128, NCH * FD], F32, tag="ladacc", name="ladacc")
            nc.vector.memset(ladacc, 0.0)

            E = spl.tile([128, NCH, 256], F32, tag="E", name="E")
            CWP = spl.tile([128, NCH, 16, 9], F32, tag="CWP", name="CWP")
            nc.vector.memset(CWP[:, :, :, 0:1], -TB)
            D = spl.tile([128, NCH, 128], F32, tag="D", name="D")

            for i in range(L - 1, -1, -1):
                flw = flwp.tile([128, 5632], BF, tag="flw", name="flw")
                nc.sync.dma_start(out=flw, in_=dram["flw"][i])
                wc0 = flw[:, 0:1024].rearrange("p (k n) -> p k n", n=256)
                wb1 = flw[:, 1024:2048].rearrange("p (j k n) -> p j k n", j=2, n=256)
                wb2 = flw[:, 2048:3072].rearrange("p (j k n) -> p j k n", j=2, n=256)
                wcb = flw[:, 3072:5120].rearrange("p (j k n) -> p j k n", j=2, n=256)
                wft = flw[:, 5120:5632].rearrange("p (k n) -> p k n", n=256)

                zT = fwork.tile([FD, BC], BF, tag="zT", name="zT", bufs=2)
                zpT = fwork.tile([FD, BC], BF, tag="zpT", name="zpT", bufs=2)
                xbm = spl.tile([128, NCH * FD], F32, tag="xbm", name="xbm", bufs=2)
                outs = spl.tile([128, NCH * FD], F32, tag=f"outs{i % 2}",
                                name="outs")

                # ---- sigma phase: all ctx gates for this layer (one
                # sigmoid-table window; PE c-matmuls overlap prev spline) ----
                scga = fwork.tile([128, 2, 2, 4, 512], BF, tag="scga",
                                  name="scga", bufs=1)
                for q in range(NBLK):
                    for j in range(NB):
                        for m in range(2):
                            cp = psum.tile([128, 512], F32, tag="ps512",
                                           name="cp", bufs=5)
                            for kt in range(4):
                                nc.tensor.matmul(
                                    cp, fr(wcb[:, j, kt, bass.ts(m, 128)]),
                                    fr(ctxT[:, kt, bass.ds(q * BLK, BLK)]),
                                    start=(kt == 0), stop=(kt == 3))
                            nc.scalar.activation(
                                scga[:, j, m, q], cp, AF.Sigmoid,
                                bias=fbpv[:, i, 6 + 6 * j + m:7 + 6 * j + m])

                for half in range(2):
                    co = half * 8            # first chunk of this half
                    for q in (2 * half, 2 * half + 1):
                        bsl = bass.ds(q * BLK, BLK)
                        # z -> zT for this block's 4 chunks
                        for c in range(4 * q, 4 * q + 4):
                            pt = psum.tile([FD, 128], F32, tag="tsp",
                                           name="tsp", bufs=1)
                            nc.tensor.transpose(pt, zbm[:, c * FD:(c + 1) * FD],
                                                ident)
                            nc.scalar.copy(zT[:, c * 128:(c + 1) * 128], pt)
                        # LU
                        pt = psum.tile([FD, BLK], F32, tag="tsp", name="tsp",
                                       bufs=1)
                        nc.tensor.matmul(pt, fr(a_sb[:, i * FD:(i + 1) * FD]),
                                         fr(zT[:, bsl]), start=True, stop=True)
                        nc.scalar.activation(zpT[:, bsl], pt, AF.Identity,
                                             bias=lub[:, i:i + 1])
                        # z' batch-major
                        for c in range(4 * q, 4 * q + 4):
                            pt = psum.tile([128, FD], BF, tag="tsp", name="tsp",
                                           bufs=1)
                            nc.tensor.transpose(pt, zpT[:, c * 128:(c + 1) * 128],
                                                ident_bf[0:FD, 0:FD])
                            nc.scalar.copy(xbm[:, c * FD:(c + 1) * FD], pt)

                        # ---- MADE ----
                        hps = []
                        for m in range(2):
                            pt = psum.tile([128, 512], F32, tag="ps512",
                                           name="hp", bufs=5)
                            nc.tensor.matmul(
                                pt, fr(w0a[:, i * H + 128 * m: i * H + 128 * (m + 1)]),
                                fr(zpT[:, bsl]), start=True, stop=False)
                            for kt in range(4):
                                nc.tensor.matmul(pt, fr(wc0[:, kt, bass.ts(m, 128)]),
                                                 fr(ctxT[:, kt, bsl]),
                                                 start=False, stop=(kt == 3))
                            hps.append(pt)
                        hsb = fwork.tile([128, 2, 512], BF, tag="hsb", name="hsb")
                        t1 = fwork.tile([128, 2, 512], BF, tag="t1", name="t1",
                                        bufs=2)
                        for m in range(2):
                            nc.scalar.activation(hsb[:, m], hps[m], AF.Identity,
                                                 bias=fbpv[:, i, m:m + 1])
                            nc.scalar.activation(t1[:, m], hps[m], AF.Relu,
                                                 bias=fbpv[:, i, m:m + 1])
                        for j in range(NB):
                            t2r = fwork.tile([128, 2, 512], BF, tag="t2r",
                                             name="t2r", bufs=2)
                            for m in range(2):
                                pt = psum.tile([128, 512], F32, tag="ps512",
                                               name="t2p", bufs=5)
                                for kt in range(2):
                                    nc.tensor.matmul(
                                        pt, fr(wb1[:, j, kt, bass.ts(m, 128)]),
                                        fr(t1[:, kt]),
                                        start=(kt == 0), stop=(kt == 1))
                                nc.scalar.activation(
                                    t2r[:, m], pt, AF.Relu,
                                    bias=fbpv[:, i, 2 + 6 * j + m:3 + 6 * j + m])
                            for m in range(2):
                                mt = fwork.tile([128, 512], BF, tag="mt",
                                                name="mt", bufs=2)
                                t3p = psum.tile([128, 512], F32, tag="ps512",
                                                name="t3p", bufs=5)
                                for kt in range(2):
                                    nc.tensor.matmul(
                                        t3p, fr(wb2[:, j, kt, bass.ts(m, 128)]),
                                        fr(t2r[:, kt]),
                                        start=(kt == 0), stop=(kt == 1))
                                nc.vector.scalar_tensor_tensor(
                                    mt, t3p, fbpv[:, i, 4 + 6 * j + m:5 + 6 * j + m],
                                    scga[:, j, m, q], ALU.add, ALU.mult)
                                nc.gpsimd.tensor_add(hsb[:, m], hsb[:, m], mt)
                                if j == 0:
                                    nc.vector.tensor_scalar_max(t1[:, m],
                                                                hsb[:, m], 0.0)
                        # params -> E via Exp
                        for c4 in range(4):
                            cg = q * 4 + c4
                            pp = psum.tile([128, 256], F32, tag="pmm", name="pmm",
                                           bufs=1)
                            for kt in range(2):
                                nc.tensor.matmul(pp, fr(hsb[:, kt, bass.ts(c4, 128)]),
                                                 fr(wft[:, kt, :]),
                                                 start=(kt == 0), stop=False)
                            nc.tensor.matmul(pp, fr(ones_rbf[0:1, :]),
                                             fr(bfr[0:1, i * 256:(i + 1) * 256]),
                                             start=False, stop=True)
                            nc.scalar.activation(E[:, cg, :], pp, AF.Exp)

                    # ---- spline for this half (8 chunks) ----
                    cs = slice(co, co + 8)
                    Ewh = E[:, cs, 0:128].rearrange("p c (g k) -> p c g k", k=K)
                    for k in range(1, K):
                        nc.vector.tensor_tensor(Ewh[:, :, :, k], Ewh[:, :, :, k],
                                                Ewh[:, :, :, k - 1], ALU.add)
                    Rt = spl.tile([128, 8, 16], F32, tag="Rt", name="Rt", bufs=2)
                    nc.vector.reciprocal(Rt, Ewh[:, :, :, K - 1])
                    nc.vector.scalar_tensor_tensor(
                        CWP[:, cs, :, 1:9], Ewh, CW_SCALE,
                        Rt[:, :, :].unsqueeze(3).broadcast_to([128, 8, 16, K]),
                        ALU.mult, ALU.mult)
                    nc.vector.tensor_tensor(
                        CWP[:, cs, :, 1:9], CWP[:, cs, :, 1:9],
                        kbias[:, :].unsqueeze(1).unsqueeze(1)
                        .broadcast_to([128, 8, 16, K]), ALU.add)
                    nc.scalar.activation(D[:, cs, :], E[:, cs, 128:256], AF.Ln,
                                         bias=1.0)
                    nc.vector.tensor_scalar_add(D[:, cs, :], D[:, cs, :], MD)

                    xvh = xbm[:, half * 64:half * 64 + 64].rearrange(
                        "p (c f) -> p c f", f=FD)
                    XC = spl.tile([128, 8, FD], F32, tag="XC", name="XC", bufs=2)
                    nc.vector.tensor_scalar(XC, xvh, TB, -TB, ALU.min, ALU.max)
                    CMPT = spl.tile([128, 8, FD, 7], F32, tag="gprod", name="CMPT",
                                    bufs=2)
                    nc.vector.tensor_tensor(
                        CMPT, XC[:, :, :].unsqueeze(3).broadcast_to([128, 8, FD, 7]),
                        CWP[:, cs, 0:8, 1:8], ALU.is_ge)
                    IDX = spl.tile([128, 8, FD], F32, tag="IDX", name="IDX", bufs=2)
                    nc.vector.tensor_reduce(IDX, CMPT, mybir.AxisListType.X, ALU.add)
                    OH = spl.tile([128, 8, FD, K], F32, tag="OH", name="OH", bufs=2)
                    nc.vector.tensor_tensor(
                        OH, IDX[:, :, :].unsqueeze(3).broadcast_to([128, 8, FD, K]),
                        kiota[:, :].unsqueeze(1).unsqueeze(1)
                        .broadcast_to([128, 8, FD, K]), ALU.is_equal)

                    def gather(src_ap, out, use_gps):
                        prod = spl.tile([128, 8, FD, K], F32, tag="gprod",
                                        name="gprod", bufs=2)
                        eng = nc.gpsimd if use_gps else nc.vector
                        eng.tensor_tensor(prod, OH, src_ap, ALU.mult)
                        nc.vector.tensor_reduce(out, prod, mybir.AxisListType.X,
                                                ALU.add)
                        return out

                    def stile(tag):
                        return spl.tile([128, 8, FD], F32, tag=tag, name=tag,
                                        bufs=2)

                    icw = gather(CWP[:, cs, 0:8, 0:8], stile("icw"), False)
                    icwR = gather(CWP[:, cs, 0:8, 1:9], stile("icwR"), True)
                    ich = gather(CWP[:, cs, 8:16, 0:8], stile("ich"), False)
                    ichR = gather(CWP[:, cs, 8:16, 1:9], stile("ichR"), True)
                    Dv = D[:, cs, :].rearrange("p c (h x) -> p c h x", h=2)
                    dk = gather(Dv[:, :, 0].rearrange("p c (f k) -> p c f k", k=K),
                                stile("dk"), False)
                    dk1 = gather(Dv[:, :, 1].rearrange("p c (f k) -> p c f k", k=K),
                                 stile("dk1"), True)

                    TT = nc.vector.tensor_tensor
                    STT = nc.vector.scalar_tensor_tensor
                    iw = stile("iw"); TT(iw, icwR, icw, ALU.subtract)
                    ih = stile("ih"); TT(ih, ichR, ich, ALU.subtract)
                    riw = stile("riw"); nc.vector.reciprocal(riw, iw)
                    delta = stile("delta"); TT(delta, ih, riw, ALU.mult)
                    tmp = stile("tmp"); TT(tmp, XC, icw, ALU.subtract)
                    th = stile("th"); TT(th, tmp, riw, ALU.mult)
                    u = stile("u"); TT(u, th, th, ALU.mult)
                    th1 = stile("th1"); TT(th1, th, u, ALU.subtract)
                    s = stile("s"); TT(s, dk, dk1, ALU.add)
                    s2 = stile("s2"); STT(s2, delta, -2.0, s, ALU.mult, ALU.add)
                    dn1 = stile("dn1"); TT(dn1, s2, th1, ALU.mult)
                    den = stile("den"); TT(den, dn1, delta, ALU.add)
                    aa = stile("aa"); TT(aa, delta, u, ALU.mult)
                    bb = stile("bb"); TT(bb, dk, th1, ALU.mult)
                    num = stile("num"); TT(num, aa, bb, ALU.add)
                    rden = stile("rden"); nc.vector.reciprocal(rden, den)
                    fr_ = stile("fr"); TT(fr_, num, rden, ALU.mult)
                    t5 = stile("t5"); TT(t5, ih, fr_, ALU.mult)
                    outsv = outs[:, half * 64:half * 64 + 64].rearrange(
                        "p (c f) -> p c f", f=FD)
                    TT(outsv, ich, t5, ALU.add)
                    vv = stile("vv"); STT(vv, th, -2.0, u, ALU.mult, ALU.add)
                    g1 = stile("g1"); TT(g1, dk1, u, ALU.mult)
                    g2 = stile("g2"); STT(g2, delta, 2.0, th1, ALU.mult, ALU.mult)
                    g3 = stile("g3"); TT(g3, dk, vv, ALU.mult)
                    i1 = stile("i1"); TT(i1, g1, g2, ALU.add)
                    i2 = stile("i2"); TT(i2, i1, g3, ALU.add)
                    inner = stile("inner"); TT(inner, i2, dk, ALU.add)
                    d2 = stile("d2"); TT(d2, delta, delta, ALU.mult)
                    dnum = stile("dnum"); TT(dnum, d2, inner, ALU.mult)
                    l1 = stile("l1")
                    nc.scalar.activation(l1, dnum, AF.Ln)
                    l2 = stile("l2")
                    nc.scalar.activation(l2, den, AF.Ln)
                    lad = stile("lad"); STT(lad, l2, -2.0, l1, ALU.mult, ALU.add)
                    xh = xbm[:, half * 64:half * 64 + 64]
                    absx = spl.tile([128, 64], F32, tag="absx", name="absx", bufs=2)
                    nc.vector.scalar_tensor_tensor(absx, xh, -1.0, xh,
                                                   ALU.mult, ALU.max)
                    msk = spl.tile([128, 64], mybir.dt.uint8, tag="msk", name="msk",
                                   bufs=2)
                    nc.vector.tensor_scalar(msk, absx, TB, None, ALU.is_gt)
                    mskv = msk[:, :].rearrange("p (c f) -> p c f", f=FD)
                    nc.vector.copy_predicated(outsv, mskv,
                                              xh.rearrange("p (c f) -> p c f", f=FD))
                    nc.vector.copy_predicated(lad, mskv,
                                              zeros128[:, 0:64].rearrange(
                                                  "p (c f) -> p c f", f=FD))
                    lav = ladacc[:, half * 64:half * 64 + 64].rearrange(
                        "p (c f) -> p c f", f=FD)
                    nc.vector.tensor_tensor(lav, lav, lad, ALU.add)
                zbm = outs  # next layer's z

            # ---- final logq ----
            zsq = spl.tile([128, NCH, FD], F32, tag="zsq", name="zsq")
            zv = zbm[:, :].rearrange("p (c f) -> p c f", f=FD)
            nc.vector.tensor_tensor(zsq, zv, zv, ALU.mult)
            zr = spl.tile([128, NCH], F32, tag="zr", name="zr")
            nc.vector.tensor_reduce(zr, zsq, mybir.AxisListType.X, ALU.add)
            lr = spl.tile([128, NCH], F32, tag="lr", name="lr")
            nc.vector.tensor_reduce(lr, ladacc[:, :].rearrange("p (c f) -> p c f", f=FD),
                                    mybir.AxisListType.X, ALU.add)
            logq = spl.tile([128, NCH], F32, tag="logq", name="logq")
            nc.vector.scalar_tensor_tensor(logq, zr, -0.5, lr, ALU.mult, ALU.add)
            nc.sync.dma_start(out=out_t[:, :], in_=logq)


_NC_CACHE = None


def _get_nc():
    global _NC_CACHE
    if _NC_CACHE is None:
        _NC_CACHE = build_nc()
    return _NC_CACHE


def kernel_logq(**inputs):
    """Full per-sample logq (device part only) — for debugging/assembly."""
    cores, host_const = prep_host(**inputs)
    nc = _get_nc()
    res = run_bass_kernel_spmd(nc, cores, list(range(NCORES)))
    parts = []
    for c in range(NCORES):
        lq = res.results[c]["logq_out"]          # [128, 16] = [p, chunk]
        parts.append(np.asarray(lq).T.reshape(BC))  # chunk-major rows
    return np.concatenate(parts), host_const


def kernel(**inputs):
    logq, host_const = kernel_logq(**inputs)
    total = float(np.mean(logq.astype(np.float64))) + host_const
    return np.float32(-total)


# revision 23
# speedup vs baseline: 1.4665x; 1.0426x over previous
"""Trainium2 Bass kernel for nn_ConditionalNSF (conditional neural spline flow NLL).

Strategy: pure data parallel over 8 NeuronCores (2048 rows each).
 - Host: tiny param prep (MADE masks, LU-fold into MADE layer 0, final-layer
   column reorder + spline boundary-derivative bias trick), final scalar mean.
 - Device, per core:
   * feature-major encoder MLP (activations [feat, batch]) with float32r
     matmuls at N=512; LayerNorm stats via PE ones-matmuls + ACT Square.
   * 4 flow layers: LU step folded into an 8x8 matmul; MADE residual MLP
     feature-major; final MADE matmul emits batch-major spline params
     (swapped operands, N=256) straight into an Exp activation.
   * rational-quadratic spline fully batch-major: cumsum via strided adds,
     bin search + one-hot gathers on DVE/GPSIMD, exp/ln/softplus on ACT.
   * per-row logq accumulated on chip, [128,16] DMA'd out per core.
"""
import sys

sys.path.insert(0, "/opt/trn_rl_repo")

import numpy as np

import concourse.bass as bass
import concourse.bacc as bacc
import concourse.tile as tile
from concourse import mybir
from concourse.bass_utils import run_bass_kernel_spmd
from concourse.masks import make_identity

AF = mybir.ActivationFunctionType
ALU = mybir.AluOpType
F32 = mybir.dt.float32
BF = mybir.dt.bfloat16

# ---- problem dims (hardcoded) ----
B = 16384
NCORES = 8
BC = B // NCORES          # 2048 rows per core
NBLK = 4                  # batch blocks of 512 per core
BLK = 512
NCH = 16                  # chunks of 128 per core
SD, AD, CTX, HID = 128, 32, 512, 512
FD, H, NB, K, L = 8, 256, 2, 8, 4
M = 3 * K - 1
TB = 6.0
MBW = MBH = MD = 1e-3
LN_EPS = 1e-5
SQH = float(np.sqrt(H))
CPAD = float(np.log(np.expm1(1.0 - MD)))
CW_SCALE = 2.0 * TB * (1.0 - MBW * K)   # cum * rec * this + kbias


def _masks():
    in_deg = np.arange(1, FD + 1)
    hid_deg = np.arange(H) % max(1, FD - 1) + min(1, FD - 1)
    m0 = (hid_deg[:, None] >= in_deg[None, :]).astype(np.float32)
    mh = (hid_deg[:, None] >= hid_deg[None, :]).astype(np.float32)
    out_deg = np.repeat(in_deg, M)
    mf = (out_deg[:, None] > hid_deg[None, :]).astype(np.float32)
    return m0, mh, mf


def _softplus(x):
    return np.log1p(np.exp(x))


def _f32(x):
    return np.ascontiguousarray(np.asarray(x, np.float32))


def _bf16(x):
    import ml_dtypes
    return np.ascontiguousarray(np.asarray(x, np.float32).astype(ml_dtypes.bfloat16))


def prep_host(state, action, x_pad, enc_params, flow_params, perms):
    """Returns (shared_inputs dict, per_core list of dicts, host_const float)."""
    p = {k: _f32(v) for k, v in enc_params.items()}
    fp = {k: _f32(v) for k, v in flow_params.items()}
    perms = np.asarray(perms)
    m0, mh, mf = _masks()

    sh = {}
    # --- encoder weights, transposed [D_in, D_out] ---
    sh["wsT"] = _f32(p["Ws"].T)                               # [128, 512]
    sh["waT"] = _f32(p["Wa"].T)                               # [32, 512]
    sh["wf1T"] = _f32(p["Wf1"].T.reshape(8, 128, 512).transpose(1, 0, 2)
                      .reshape(128, 8 * 512))                 # [128, kt*512]
    for nm, tag in (("Wf2", "wf2T"), ("Wo1", "wo1T"), ("Wo2", "wo2T")):
        sh[tag] = _f32(p[nm].T.reshape(4, 128, 512).transpose(1, 0, 2)
                       .reshape(128, 4 * 512))
    # encoder biases [128, 6, 4] (slot l, chunk m)
    encbp = np.zeros((128, 6, 4), np.float32)
    for l, nm in enumerate(["bs", "ba", "bf1", "bf2", "bo1", "bo2"]):
        encbp[:, l, :] = p[nm].reshape(4, 128).T
    sh["encbp"] = _f32(encbp.reshape(128, 24))
    encg = np.zeros((128, 5, 4), np.float32)
    encbg = np.zeros((128, 5, 4), np.float32)
    for l, (g, bg) in enumerate([("gs", "bgs"), ("ga", "bga"), ("gf1", "bgf1"),
                                 ("gf2", "bgf2"), ("go1", "bgo1")]):
        encg[:, l, :] = p[g].reshape(4, 128).T
        encbg[:, l, :] = p[bg].reshape(4, 128).T
    sh["encg"] = _f32(encg.reshape(128, 20))
    sh["encbg"] = _f32(encbg.reshape(128, 20))

    # --- flow prep ---
    A = np.zeros((L, FD, FD), np.float32)
    ld_host = 0.0
    for i in range(L):
        Pm = np.zeros((FD, FD), np.float32)
        Pm[perms[i], np.arange(FD)] = 1.0
        lw = np.tril(fp["lu_L"][i], -1) + np.eye(FD, dtype=np.float32)
        udiag = _softplus(fp["lu_d"][i]) + 1e-3
        up = np.triu(fp["lu_U"][i], 1) + np.diag(udiag)
        A[i] = Pm @ up.T @ lw.T
        ld_host += float(np.sum(np.log(udiag)))
    sh["a_sb"] = _f32(A.transpose(1, 0, 2).reshape(FD, L * FD))   # [8, L*8]
    sh["lub"] = _f32(fp["lu_b"].T)                                # [8, L]

    W0m = fp["W0"] * m0[None]
    w0a = np.stack([A[i] @ W0m[i].T for i in range(L)])           # [L, 8, H]
    sh["w0a"] = _f32(w0a.transpose(1, 0, 2).reshape(FD, L * H))   # [8, L*256]
    bias0 = np.stack([fp["lu_b"][i] @ W0m[i].T + fp["b0"][i] + fp["bc0"][i]
                      for i in range(L)])                         # [L, H]

    # per-partition bias pack [128, L, 14]:
    # slots: 0-1 bias0(m), then per j (j*6): 2+6j bb1(m), 4+6j bb2(m)(UNUSED,
    # bb2 folded into mt STT), ... keep simple: 0-1 bias0, 2-3 bb1 j0,
    # 4-5 bb2 j0, 6-7 bcb j0, 8-9 bb1 j1, 10-11 bb2 j1, 12-13 bcb j1
    fbp = np.zeros((128, L, 14), np.float32)
    for i in range(L):
        fbp[:, i, 0:2] = bias0[i].reshape(2, 128).T
        for j in range(NB):
            fbp[:, i, 2 + 6 * j:4 + 6 * j] = fp["bb1"][i, j].reshape(2, 128).T
            fbp[:, i, 4 + 6 * j:6 + 6 * j] = fp["bb2"][i, j].reshape(2, 128).T
            fbp[:, i, 6 + 6 * j:8 + 6 * j] = fp["bcb"][i, j].reshape(2, 128).T
    sh["fbp"] = _f32(fbp.reshape(128, L * 14))

    # final-layer rhs [H, 256] per layer: cols [UW(64)|UH(64)|D0(64)|D1(64)]
    Wfm = fp["Wf"] * mf[None]
    bf = fp["bf"]
    wft = np.zeros((L, H, 256), np.float32)
    bfr = np.zeros((L, 256), np.float32)
    for i in range(L):
        for f in range(FD):
            for k in range(K):
                wft[i, :, f * K + k] = Wfm[i, f * M + k] / SQH
                bfr[i, f * K + k] = bf[i, f * M + k] / SQH
                wft[i, :, 64 + f * K + k] = Wfm[i, f * M + K + k] / SQH
                bfr[i, 64 + f * K + k] = bf[i, f * M + K + k] / SQH
                if k == 0:
                    bfr[i, 128 + f * K] = CPAD
                else:
                    wft[i, :, 128 + f * K + k] = Wfm[i, f * M + 2 * K + k - 1]
                    bfr[i, 128 + f * K + k] = bf[i, f * M + 2 * K + k - 1]
                if k == K - 1:
                    bfr[i, 192 + f * K + k] = CPAD
                else:
                    wft[i, :, 192 + f * K + k] = Wfm[i, f * M + 2 * K + k]
                    bfr[i, 192 + f * K + k] = bf[i, f * M + 2 * K + k]
    sh["bfr"] = _f32(bfr.reshape(1, L * 256))

    # streamed per-layer weight pack [L, 128, 5632]:
    # cols: wc0 (kt4,256)=0:1024 | wb1 (j2,kt2,256)=1024:2048 |
    #       wb2 (j2,kt2,256)=2048:3072 | wcb (j2,kt4,256)=3072:5120 |
    #       wft (kt2,256)=5120:5632
    flw = np.zeros((L, 128, 5632), np.float32)
    for i in range(L):
        wc0T = fp["Wc0"][i].T            # [CTX, H]
        flw[i, :, 0:1024] = wc0T.reshape(4, 128, 256).transpose(1, 0, 2).reshape(128, 1024)
        for j in range(NB):
            b1 = (fp["Wb1"][i, j] * mh).T    # [H, H]
            b2 = (fp["Wb2"][i, j] * mh).T
            cb = fp["Wcb"][i, j].T           # [CTX, H]
            flw[i, :, 1024 + 512 * j:1024 + 512 * (j + 1)] = \
                b1.reshape(2, 128, 256).transpose(1, 0, 2).reshape(128, 512)
            flw[i, :, 2048 + 512 * j:2048 + 512 * (j + 1)] = \
                b2.reshape(2, 128, 256).transpose(1, 0, 2).reshape(128, 512)
            flw[i, :, 3072 + 1024 * j:3072 + 1024 * (j + 1)] = \
                cb.reshape(4, 128, 256).transpose(1, 0, 2).reshape(128, 1024)
        flw[i, :, 5120:5632] = wft[i].reshape(2, 128, 256).transpose(1, 0, 2).reshape(128, 512)
    sh["flw"] = _f32(flw)

    # spline consts
    kk = np.arange(1, K + 1, dtype=np.float32)
    sh["kbias"] = _f32(np.broadcast_to(2 * TB * MBW * kk - TB, (128, K)))
    sh["kiota"] = _f32(np.broadcast_to(np.arange(K, dtype=np.float32), (128, K)))

    for k in ["a_sb", "w0a", "bfr", "flw"]:
        sh[k] = _bf16(sh[k])

    # single packed bf16 tensor for ALL encoder-phase DMAs (one DMA proc ->
    # small released-zone dep sets). col layout:
    # state 0:2048 | action 2048:4096 (rows<32) | wsT 4096:4608 |
    # waT 4608:5120 (rows<32) | wf1T 5120:9216 | wf2T 9216:11264 |
    # wo1T 11264:13312 | wo2T 13312:15360
    import ml_dtypes
    encpack0 = np.zeros((128, 15360), dtype=ml_dtypes.bfloat16)
    encpack0[:, 4096:4608] = _bf16(sh.pop("wsT"))
    encpack0[0:AD, 4608:5120] = _bf16(sh.pop("waT"))
    encpack0[:, 5120:9216] = _bf16(sh.pop("wf1T"))
    encpack0[:, 9216:11264] = _bf16(sh.pop("wf2T"))
    encpack0[:, 11264:13312] = _bf16(sh.pop("wo1T"))
    encpack0[:, 13312:15360] = _bf16(sh.pop("wo2T"))

    # --- per-core data ---
    state = _f32(state)
    action = _f32(action)
    x_pad = _f32(x_pad)
    cores = []
    for c in range(NCORES):
        s = state[c * BC:(c + 1) * BC]
        a = action[c * BC:(c + 1) * BC]
        x = x_pad[c * BC:(c + 1) * BC]
        d = dict(sh)
        ep = encpack0.copy()
        ep[:, 0:2048] = _bf16(s.T)
        ep[0:AD, 2048:4096] = _bf16(a.T)
        d["encpack"] = ep
        # batch-major [128, c*8+f]
        d["xpbm"] = _f32(x.reshape(NCH, 128, FD).transpose(1, 0, 2).reshape(128, NCH * FD))
        cores.append(d)

    host_const = ld_host - 0.5 * FD * float(np.log(2.0 * np.pi))
    return cores, host_const


# ---------------------------------------------------------------------------
# device program
# ---------------------------------------------------------------------------

def fr(ap):
    return ap


def build_nc():
    nc = bacc.Bacc("TRN2", target_bir_lowering=False, debug=False)
    dram = {}

    BF_NAMES = {"encpack", "a_sb", "w0a", "bfr", "flw"}

    def din(name, shape):
        dt = BF if name in BF_NAMES else F32
        dram[name] = nc.declare_dram_parameter(name, list(shape), dt, isOutput=False)
        return dram[name]

    din("encpack", (128, 15360))
    din("xpbm", (128, NCH * FD))
    din("encbp", (128, 24))
    din("encg", (128, 20))
    din("encbg", (128, 20))
    din("a_sb", (FD, L * FD))
    din("lub", (FD, L))
    din("w0a", (FD, L * H))
    din("fbp", (128, L * 14))
    din("bfr", (1, L * 256))
    din("flw", (L, 128, 5632))
    din("kbias", (128, K))
    din("kiota", (128, K))
    out_t = nc.declare_dram_parameter("logq_out", [128, NCH], F32, isOutput=True)

    with tile.TileContext(nc) as tc:
        _body(nc, tc, dram, out_t)
    nc.compile()
    return nc


def _body(nc, tc, dram, out_t):
    from contextlib import ExitStack
    ctx = ExitStack()
    with ctx:
        const = ctx.enter_context(tc.tile_pool(name="const", bufs=1))
        persist = ctx.enter_context(tc.tile_pool(name="persist", bufs=1))


        # ---- constants ----
        ident = const.tile([128, 128], F32, tag="ident", name="ident")
        make_identity(nc, ident)
        ones_col = const.tile([128, 1], BF, tag="ones_col", name="ones_col")
        nc.vector.memset(ones_col, 1.0)
        ones_row = const.tile([1, 512], F32, tag="ones_row", name="ones_row")
        nc.vector.memset(ones_row, 1.0)
        ones_rbf = const.tile([1, 128], BF, tag="ones_rbf", name="ones_rbf")
        nc.vector.memset(ones_rbf, 1.0)
        ident_bf = const.tile([128, 128], BF, tag="ident_bf", name="ident_bf")
        make_identity(nc, ident_bf)
        zeros128 = const.tile([128, 128], F32, tag="zeros128", name="zeros128")
        nc.vector.memset(zeros128, 0.0)
        kbias = const.tile([128, K], F32, tag="kbias", name="kbias")
        nc.sync.dma_start(out=kbias, in_=dram["kbias"][:, :])
        kiota = const.tile([128, K], F32, tag="kiota", name="kiota")
        nc.sync.dma_start(out=kiota, in_=dram["kiota"][:, :])
        epsc = const.tile([128, 1], F32, tag="epsc", name="epsc")
        nc.vector.memset(epsc, LN_EPS)

        # ---- persistent weights/data ----
        def load(name, shape, dt=F32):
            t = persist.tile(list(shape), dt, tag=name, name=name)
            nc.sync.dma_start(out=t, in_=dram[name][:, :])
            return t

        encbp = load("encbp", (128, 24))
        encg = load("encg", (128, 20))
        encbg = load("encbg", (128, 20))
        a_sb = load("a_sb", (FD, L * FD), BF)
        lub = load("lub", (FD, L))
        w0a = load("w0a", (FD, L * H), BF)
        fbp = load("fbp", (128, L * 14))
        bfr = load("bfr", (1, L * 256), BF)

        ctxT = persist.tile([128, 4, BC], BF, tag="ctxT", name="ctxT")       # encoder out

        # =========================== ENCODER ===========================
        with tc.tile_pool(name="encw", bufs=1) as encw, \
             tc.tile_pool(name="ework", bufs=2) as ework, \
             tc.tile_pool(name="psum_e", bufs=2, space="PSUM") as psum:

            encpack = encw.tile([128, 15360], BF, tag="encpack", name="encpack")
            nc.sync.dma_start(out=encpack, in_=dram["encpack"][:, :])
            stateT = encpack[:, 0:2048]
            actionT = encpack[0:AD, 2048:4096]
            wsT = encpack[:, 4096:4608]
            waT = encpack[0:AD, 4608:5120]
            wf1T = encpack[:, 5120:9216].rearrange("p (k n) -> p k n", n=512)
            wf2T = encpack[:, 9216:11264].rearrange("p (k n) -> p k n", n=512)
            wo1T = encpack[:, 11264:13312].rearrange("p (k n) -> p k n", n=512)
            wo2T = encpack[:, 13312:15360].rearrange("p (k n) -> p k n", n=512)

            encgv = encg[:, :].rearrange("p (l m) -> p l m", m=4)
            encbgv = encbg[:, :].rearrange("p (l m) -> p l m", m=4)
            encbpv = encbp[:, :].rearrange("p (l m) -> p l m", m=4)

            def mm_layer(rhs_fn, nkt, w_ap_fn, out_psums):
                """4 accumulating matmuls per out-chunk m."""
                for m in range(4):
                    pt = out_psums[m]
                    for kt in range(nkt):
                        nc.tensor.matmul(pt, fr(w_ap_fn(kt, m)), fr(rhs_fn(kt)),
                                         start=(kt == 0), stop=(kt == nkt - 1))

            def ln_relu(q, yps, lidx, bslot, out_tile):
                """LayerNorm+ReLU, feature-major, block q. yps: 4 psum [128,512].

                rstd computed as exp(-0.5*ln(var+eps)) to stay in the
                natural_log_exp ACT table set and avoid DVE iterative divide.
                """
                ysb = ework.tile([128, 4, 512], BF, tag="ysb", name="ysb", bufs=3)
                sq = ework.tile([128, 4, 512], BF, tag="sq", name="sq", bufs=2)
                for m in range(4):
                    nc.scalar.activation(ysb[:, m], yps[m], AF.Identity,
                                         bias=encbpv[:, bslot, m:m + 1])
                    nc.vector.tensor_tensor(sq[:, m], ysb[:, m], ysb[:, m], ALU.mult)
                sps = psum.tile([33, 512], F32, tag="srow_ps", name="srow_ps", bufs=1)
                for m in range(4):
                    nc.tensor.matmul(sps[0:1, :], fr(ones_col[:, 0:1]), fr(ysb[:, m]),
                                     start=(m == 0), stop=(m == 3))
                for m in range(4):
                    nc.tensor.matmul(sps[32:33, :], fr(ones_col[:, 0:1]), fr(sq[:, m]),
                                     start=(m == 0), stop=(m == 3))
                srow = ework.tile([1, 512], F32, tag="srow", name="srow", bufs=2)
                nc.scalar.copy(srow, sps[0:1])
                srow2 = ework.tile([1, 512], F32, tag="srow2", name="srow2", bufs=2)
                nc.scalar.copy(srow2, sps[32:33])
                musq = ework.tile([1, 512], F32, tag="musq", name="musq", bufs=2)
                nc.scalar.activation(musq, srow, AF.Square, scale=1.0 / 512.0)
                varr = ework.tile([1, 512], F32, tag="varr", name="varr", bufs=2)
                nc.vector.scalar_tensor_tensor(varr, srow2, 1.0 / 512.0, musq,
                                               ALU.mult, ALU.subtract)
                lnv = ework.tile([1, 512], F32, tag="lnv", name="lnv", bufs=2)
                nc.scalar.activation(lnv, varr, AF.Ln, bias=epsc[0:1, 0:1])
                rstd = ework.tile([1, 512], F32, tag="rstd", name="rstd", bufs=2)
                nc.scalar.activation(rstd, lnv, AF.Exp, scale=-0.5)
                nmrs = ework.tile([1, 512], F32, tag="nmrs", name="nmrs", bufs=2)
                nc.vector.scalar_tensor_tensor(nmrs, srow, -1.0 / 512.0, rstd,
                                               ALU.mult, ALU.mult)
                rstdbc = psum.tile([128, 512], F32, tag="ps512", name="rstdbc", bufs=7)
                nc.tensor.matmul(rstdbc, fr(ones_row[0:1, 0:128]), fr(rstd),
                                 start=True, stop=True)
                nmrsbc = psum.tile([128, 512], F32, tag="ps512", name="nmrsbc", bufs=7)
                nc.tensor.matmul(nmrsbc, fr(ones_row[0:1, 0:128]), fr(nmrs),
                                 start=True, stop=True)
                for m in range(4):
                    g_ap = encgv[:, lidx, m:m + 1]
                    v = ework.tile([128, 512], F32, tag="v", name="v", bufs=3)
                    w = ework.tile([128, 512], F32, tag="w", name="w", bufs=3)
                    nc.vector.scalar_tensor_tensor(v, ysb[:, m], g_ap, rstdbc,
                                                   ALU.mult, ALU.mult)
                    nc.vector.scalar_tensor_tensor(w, nmrsbc, g_ap, v,
                                                   ALU.mult, ALU.add)
                    nc.scalar.activation(out_tile[:, m], w, AF.Relu,
                                         bias=encbgv[:, lidx, m:m + 1])

            # layer-major: all 4 blocks per layer so PE pipelines across
            # each block's LayerNorm latency.
            s1 = [None] * NBLK; a1 = [None] * NBLK
            h2 = [None] * NBLK; h3 = [None] * NBLK; h4 = [None] * NBLK

            def enc_layer(out_list, nkt, rhs_fn, w_fn, lidx, bslot):
                for q in range(NBLK):
                    yps = [psum.tile([128, 512], F32, tag="ps512", name="ps512",
                                     bufs=7) for _ in range(4)]
                    mm_layer(lambda kt: rhs_fn(kt, q), nkt, w_fn, yps)
                    t = ework.tile([128, 4, 512], BF, tag="hact", name="hact",
                                   bufs=12)
                    ln_relu(q, yps, lidx, bslot, t)
                    out_list[q] = t

            enc_layer(s1, 1, lambda kt, q: stateT[:, bass.ds(q * BLK, BLK)],
                      lambda kt, m: wsT[:, bass.ts(m, 128)], 0, 0)
            enc_layer(a1, 1, lambda kt, q: actionT[:, bass.ds(q * BLK, BLK)],
                      lambda kt, m: waT[:, bass.ts(m, 128)], 1, 1)
            enc_layer(h2, 8, lambda kt, q: s1[q][:, kt] if kt < 4 else a1[q][:, kt - 4],
                      lambda kt, m: wf1T[:, kt, bass.ts(m, 128)], 2, 2)
            enc_layer(h3, 4, lambda kt, q: h2[q][:, kt],
                      lambda kt, m: wf2T[:, kt, bass.ts(m, 128)], 3, 3)
            enc_layer(h4, 4, lambda kt, q: h3[q][:, kt],
                      lambda kt, m: wo1T[:, kt, bass.ts(m, 128)], 4, 4)
            for q in range(NBLK):
                yps = [psum.tile([128, 512], F32, tag="ps512", name="ps512",
                                 bufs=7) for _ in range(4)]
                mm_layer(lambda kt: h4[q][:, kt], 4,
                         lambda kt, m: wo2T[:, kt, bass.ts(m, 128)], yps)
                for m in range(4):
                    nc.scalar.activation(ctxT[:, m, bass.ds(q * BLK, BLK)], yps[m],
                                         AF.Identity, bias=encbpv[:, 5, m:m + 1])

        # =========================== FLOWS ===========================
        with tc.tile_pool(name="flw", bufs=2) as flwp, \
             tc.tile_pool(name="spl", bufs=1) as spl, \
             tc.tile_pool(name="fwork", bufs=2) as fwork, \
             tc.tile_pool(name="psum_f", bufs=2, space="PSUM") as psum:

            fbpv = fbp[:, :].rearrange("p (l s) -> p l s", s=14)

            zbm = spl.tile([128, NCH * FD], F32, tag="zbm", name="zbm")
            nc.sync.dma_start(out=zbm, in_=dram["xpbm"][:, :])
            ladacc = spl.tile([128, NCH * FD], F32, tag="ladacc", name="ladacc")
            nc.vector.memset(ladacc, 0.0)

            E = spl.tile([128, NCH, 256], F32, tag="E", name="E")
            CWP = spl.tile([128, NCH, 16, 9], F32, tag="CWP", name="CWP")
            nc.vector.memset(CWP[:, :, :, 0:1], -TB)
            D = spl.tile([128, NCH, 128], F32, tag="D", name="D")

            for i in range(L - 1, -1, -1):
                flw = flwp.tile([128, 5632], BF, tag="flw", name="flw")
                nc.sync.dma_start(out=flw, in_=dram["flw"][i])
                wc0 = flw[:, 0:1024].rearrange("p (k n) -> p k n", n=256)
                wb1 = flw[:, 1024:2048].rearrange("p (j k n) -> p j k n", j=2, n=256)
                wb2 = flw[:, 2048:3072].rearrange("p (j k n) -> p j k n", j=2, n=256)
                wcb = flw[:, 3072:5120].rearrange("p (j k n) -> p j k n", j=2, n=256)
                wft = flw[:, 5120:5632].rearrange("p (k n) -> p k n", n=256)

                zT = fwork.tile([FD, BC], BF, tag="zT", name="zT", bufs=2)
                zpT = fwork.tile([FD, BC], BF, tag="zpT", name="zpT", bufs=2)
                xbm = spl.tile([128, NCH * FD], F32, tag="xbm", name="xbm", bufs=2)
                outs = spl.tile([128, NCH * FD], F32, tag=f"outs{i % 2}",
                                name="outs")

                # ---- sigma phase: all ctx gates for this layer (one
                # sigmoid-table window; PE c-matmuls overlap prev spline) ----
                scga = fwork.tile([128, 2, 2, 4, 512], BF, tag="scga",
                                  name="scga", bufs=1)
                for q in range(NBLK):
                    for j in range(NB):
                        for m in range(2):
                            cp = psum.tile([128, 512], F32, tag="ps512",
                                           name="cp", bufs=5)
                            for kt in range(4):
                                nc.tensor.matmul(
                                    cp, fr(wcb[:, j, kt, bass.ts(m, 128)]),
                                    fr(ctxT[:, kt, bass.ds(q * BLK, BLK)]),
                                    start=(kt == 0), stop=(kt == 3))
                            nc.scalar.activation(
                                scga[:, j, m, q], cp, AF.Sigmoid,
                                bias=fbpv[:, i, 6 + 6 * j + m:7 + 6 * j + m])

                for half in range(2):
                    co = half * 8            # first chunk of this half
                    for q in (2 * half, 2 * half + 1):
                        bsl = bass.ds(q * BLK, BLK)
                        # z -> zT for this block's 4 chunks
                        for c in range(4 * q, 4 * q + 4):
                            pt = psum.tile([FD, 128], F32, tag="tsp",
                                           name="tsp", bufs=1)
                            nc.tensor.transpose(pt, zbm[:, c * FD:(c + 1) * FD],
                                                ident)
                            nc.scalar.copy(zT[:, c * 128:(c + 1) * 128], pt)
                        # LU
                        pt = psum.tile([FD, BLK], F32, tag="tsp", name="tsp",
                                       bufs=1)
                        nc.tensor.matmul(pt, fr(a_sb[:, i * FD:(i + 1) * FD]),
                                         fr(zT[:, bsl]), start=True, stop=True)
                        nc.scalar.activation(zpT[:, bsl], pt, AF.Identity,
                                             bias=lub[:, i:i + 1])
                        # z' batch-major
                        for c in range(4 * q, 4 * q + 4):
                            pt = psum.tile([128, FD], BF, tag="tsp", name="tsp",
                                           bufs=1)
                            nc.tensor.transpose(pt, zpT[:, c * 128:(c + 1) * 128],
                                                ident_bf[0:FD, 0:FD])
                            nc.scalar.copy(xbm[:, c * FD:(c + 1) * FD], pt)

                        # ---- MADE ----
                        hps = []
                        for m in range(2):
                            pt = psum.tile([128, 512], F32, tag="ps512",
                                           name="hp", bufs=5)
                            nc.tensor.matmul(
                                pt, fr(w0a[:, i * H + 128 * m: i * H + 128 * (m + 1)]),
                                fr(zpT[:, bsl]), start=True, stop=False)
                            for kt in range(4):
                                nc.tensor.matmul(pt, fr(wc0[:, kt, bass.ts(m, 128)]),
                                                 fr(ctxT[:, kt, bsl]),
                                                 start=False, stop=(kt == 3))
                            hps.append(pt)
                        hsb = fwork.tile([128, 2, 512], BF, tag="hsb", name="hsb")
                        t1 = fwork.tile([128, 2, 512], BF, tag="t1", name="t1",
                                        bufs=2)
                        for m in range(2):
                            nc.scalar.activation(hsb[:, m], hps[m], AF.Identity,
                                                 bias=fbpv[:, i, m:m + 1])
                            nc.scalar.activation(t1[:, m], hps[m], AF.Relu,
                                                 bias=fbpv[:, i, m:m + 1])
                        for j in range(NB):
                            t2r = fwork.tile([128, 2, 512], BF, tag="t2r",
                                             name="t2r", bufs=2)
                            for m in range(2):
                                pt = psum.tile([128, 512], F32, tag="ps512",
                                               name="t2p", bufs=5)
                                for kt in range(2):
                                    nc.tensor.matmul(
                                        pt, fr(wb1[:, j, kt, bass.ts(m, 128)]),
                                        fr(t1[:, kt]),
                                        start=(kt == 0), stop=(kt == 1))
                                nc.scalar.activation(
                                    t2r[:, m], pt, AF.Relu,
                                    bias=fbpv[:, i, 2 + 6 * j + m:3 + 6 * j + m])
                            for m in range(2):
                                mt = fwork.tile([128, 512], BF, tag="mt",
                                                name="mt", bufs=2)
                                t3p = psum.tile([128, 512], F32, tag="ps512",
                                                name="t3p", bufs=5)
                                for kt in range(2):
                                    nc.tensor.matmul(
                                        t3p, fr(wb2[:, j, kt, bass.ts(m, 128)]),
                                        fr(t2r[:, kt]),
                                        start=(kt == 0), stop=(kt == 1))
                                nc.vector.scalar_tensor_tensor(
                                    mt, t3p, fbpv[:, i, 4 + 6 * j + m:5 + 6 * j + m],
                                    scga[:, j, m, q], ALU.add, ALU.mult)
                                nc.gpsimd.tensor_add(hsb[:, m], hsb[:, m], mt)
                                if j == 0:
                                    nc.vector.tensor_scalar_max(t1[:, m],
                                                                hsb[:, m], 0.0)
                        # params -> E via Exp
                        for c4 in range(4):
                            cg = q * 4 + c4
                            pp = psum.tile([128, 256], F32, tag="pmm", name="pmm",
                                           bufs=1)
                            for kt in range(2):
                                nc.tensor.matmul(pp, fr(hsb[:, kt, bass.ts(c4, 128)]),
                                                 fr(wft[:, kt, :]),
                                                 start=(kt == 0), stop=False)
                            nc.tensor.matmul(pp, fr(ones_rbf[0:1, :]),
                                             fr(bfr[0:1, i * 256:(i + 1) * 256]),
                                             start=False, stop=True)
                            nc.scalar.activation(E[:, cg, :], pp, AF.Exp)

                    # ---- spline for this half (8 chunks) ----
                    cs = slice(co, co + 8)
                    Ewh = E[:, cs, 0:128].rearrange("p c (g k) -> p c g k", k=K)
                    for k in range(1, K):
                        nc.vector.tensor_tensor(Ewh[:, :, :, k], Ewh[:, :, :, k],
                                                Ewh[:, :, :, k - 1], ALU.add)
                    Rt = spl.tile([128, 8, 16], F32, tag="Rt", name="Rt", bufs=2)
                    nc.vector.reciprocal(Rt, Ewh[:, :, :, K - 1])
                    nc.vector.scalar_tensor_tensor(
                        CWP[:, cs, :, 1:9], Ewh, CW_SCALE,
                        Rt[:, :, :].unsqueeze(3).broadcast_to([128, 8, 16, K]),
                        ALU.mult, ALU.mult)
                    nc.vector.tensor_tensor(
                        CWP[:, cs, :, 1:9], CWP[:, cs, :, 1:9],
                        kbias[:, :].unsqueeze(1).unsqueeze(1)
                        .broadcast_to([128, 8, 16, K]), ALU.add)
                    nc.scalar.activation(D[:, cs, :], E[:, cs, 128:256], AF.Ln,
                                         bias=1.0)
                    nc.vector.tensor_scalar_add(D[:, cs, :], D[:, cs, :], MD)

                    xvh = xbm[:, half * 64:half * 64 + 64].rearrange(
                        "p (c f) -> p c f", f=FD)
                    XC = spl.tile([128, 8, FD], F32, tag="XC", name="XC", bufs=2)
                    nc.vector.tensor_scalar(XC, xvh, TB, -TB, ALU.min, ALU.max)
                    CMPT = spl.tile([128, 8, FD, 7], F32, tag="gprod", name="CMPT",
                                    bufs=2)
                    nc.vector.tensor_tensor(
                        CMPT, XC[:, :, :].unsqueeze(3).broadcast_to([128, 8, FD, 7]),
                        CWP[:, cs, 0:8, 1:8], ALU.is_ge)
                    IDX = spl.tile([128, 8, FD], F32, tag="IDX", name="IDX", bufs=2)
                    nc.vector.tensor_reduce(IDX, CMPT, mybir.AxisListType.X, ALU.add)
                    OH = spl.tile([128, 8, FD, K], F32, tag="OH", name="OH", bufs=2)
                    nc.vector.tensor_tensor(
                        OH, IDX[:, :, :].unsqueeze(3).broadcast_to([128, 8, FD, K]),
                        kiota[:, :].unsqueeze(1).unsqueeze(1)
                        .broadcast_to([128, 8, FD, K]), ALU.is_equal)

                    def gather(src_ap, out, use_gps):
                        prod = spl.tile([128, 8, FD, K], F32, tag="gprod",
                                        name="gprod", bufs=2)
                        eng = nc.gpsimd if use_gps else nc.vector
                        eng.tensor_tensor(prod, OH, src_ap, ALU.mult)
                        nc.vector.tensor_reduce(out, prod, mybir.AxisListType.X,
                                                ALU.add)
                        return out

                    def stile(tag):
                        return spl.tile([128, 8, FD], F32, tag=tag, name=tag,
                                        bufs=2)

                    icw = gather(CWP[:, cs, 0:8, 0:8], stile("icw"), False)
                    icwR = gather(CWP[:, cs, 0:8, 1:9], stile("icwR"), True)
                    ich = gather(CWP[:, cs, 8:16, 0:8], stile("ich"), False)
                    ichR = gather(CWP[:, cs, 8:16, 1:9], stile("ichR"), True)
                    Dv = D[:, cs, :].rearrange("p c (h x) -> p c h x", h=2)
                    dk = gather(Dv[:, :, 0].rearrange("p c (f k) -> p c f k", k=K),
                                stile("dk"), False)
                    dk1 = gather(Dv[:, :, 1].rearrange("p c (f k) -> p c f k", k=K),
                                 stile("dk1"), True)

                    TT = nc.vector.tensor_tensor
                    STT = nc.vector.scalar_tensor_tensor
                    iw = stile("iw"); TT(iw, icwR, icw, ALU.subtract)
                    ih = stile("ih"); TT(ih, ichR, ich, ALU.subtract)
                    riw = stile("riw"); nc.vector.reciprocal(riw, iw)
                    delta = stile("delta"); TT(delta, ih, riw, ALU.mult)
                    tmp = stile("tmp"); TT(tmp, XC, icw, ALU.subtract)
                    th = stile("th"); TT(th, tmp, riw, ALU.mult)
                    u = stile("u"); TT(u, th, th, ALU.mult)
                    th1 = stile("th1"); TT(th1, th, u, ALU.subtract)
                    s = stile("s"); TT(s, dk, dk1, ALU.add)
                    s2 = stile("s2"); STT(s2, delta, -2.0, s, ALU.mult, ALU.add)
                    dn1 = stile("dn1"); TT(dn1, s2, th1, ALU.mult)
                    den = stile("den"); TT(den, dn1, delta, ALU.add)
                    aa = stile("aa"); TT(aa, delta, u, ALU.mult)
                    bb = stile("bb"); TT(bb, dk, th1, ALU.mult)
                    num = stile("num"); TT(num, aa, bb, ALU.add)
                    rden = stile("rden"); nc.vector.reciprocal(rden, den)
                    fr_ = stile("fr"); TT(fr_, num, rden, ALU.mult)
                    t5 = stile("t5"); TT(t5, ih, fr_, ALU.mult)
                    outsv = outs[:, half * 64:half * 64 + 64].rearrange(
                        "p (c f) -> p c f", f=FD)
                    TT(outsv, ich, t5, ALU.add)
                    vv = stile("vv"); STT(vv, th, -2.0, u, ALU.mult, ALU.add)
                    g1 = stile("g1"); TT(g1, dk1, u, ALU.mult)
                    g2 = stile("g2"); STT(g2, delta, 2.0, th1, ALU.mult, ALU.mult)
                    g3 = stile("g3"); TT(g3, dk, vv, ALU.mult)
                    i1 = stile("i1"); TT(i1, g1, g2, ALU.add)
                    i2 = stile("i2"); TT(i2, i1, g3, ALU.add)
                    inner = stile("inner"); TT(inner, i2, dk, ALU.add)
                    d2 = stile("d2"); TT(d2, delta, delta, ALU.mult)
                    dnum = stile("dnum"); TT(dnum, d2, inner, ALU.mult)
                    l1 = stile("l1")
                    nc.scalar.activation(l1, dnum, AF.Ln)
                    l2 = stile("l2")
                    nc.scalar.activation(l2, den, AF.Ln)
                    lad = stile("lad"); STT(lad, l2, -2.0, l1, ALU.mult, ALU.add)
                    xh = xbm[:, half * 64:half * 64 + 64]
                    absx = spl.tile([128, 64], F32, tag="absx", name="absx", bufs=2)
                    nc.vector.scalar_tensor_tensor(absx, xh, -1.0, xh,
                                                   ALU.mult, ALU.max)
                    msk = spl.tile([128, 64], mybir.dt.uint8, tag="msk", name="msk",
                                   bufs=2)
                    nc.vector.tensor_scalar(msk, absx, TB, None, ALU.is_gt)
                    mskv = msk[:, :].rearrange("p (c f) -> p c f", f=FD)
                    nc.vector.copy_predicated(outsv, mskv,
                                              xh.rearrange("p (c f) -> p c f", f=FD))
                    nc.vector.copy_predicated(lad, mskv,
                                              zeros128[:, 0:64].rearrange(
                                                  "p (c f) -> p c f", f=FD))
                    lav = ladacc[:, half * 64:half * 64 + 64].rearrange(
                        "p (c f) -> p c f", f=FD)
                    nc.vector.tensor_tensor(lav, lav, lad, ALU.add)
                zbm = outs  # next layer's z

            # ---- final logq ----
            zsq = spl.tile([128, NCH, FD], F32, tag="zsq", name="zsq")
            zv = zbm[:, :].rearrange("p (c f) -> p c f", f=FD)
            nc.vector.tensor_tensor(zsq, zv, zv, ALU.mult)
            zr = spl.tile([128, NCH], F32, tag="zr", name="zr")
            nc.vector.tensor_reduce(zr, zsq, mybir.AxisListType.X, ALU.add)
            lr = spl.tile([128, NCH], F32, tag="lr", name="lr")
            nc.vector.tensor_reduce(lr, ladacc[:, :].rearrange("p (c f) -> p c f", f=FD),
                                    mybir.AxisListType.X, ALU.add)
            logq = spl.tile([128, NCH], F32, tag="logq", name="logq")
            nc.vector.scalar_tensor_tensor(logq, zr, -0.5, lr, ALU.mult, ALU.add)
            nc.sync.dma_start(out=out_t[:, :], in_=logq)


_NC_CACHE = None


def _get_nc():
    global _NC_CACHE
    if _NC_CACHE is None:
        _NC_CACHE = build_nc()
    return _NC_CACHE


def kernel_logq(**inputs):
    """Full per-sample logq (device part only) — for debugging/assembly."""
    cores, host_const = prep_host(**inputs)
    nc = _get_nc()
    res = run_bass_kernel_spmd(nc, cores, list(range(NCORES)))
    parts = []
    for c in range(NCORES):
        lq = res.results[c]["logq_out"]          # [128, 16] = [p, chunk]
        parts.append(np.asarray(lq).T.reshape(BC))  # chunk-major rows
    return np.concatenate(parts), host_const


def kernel(**inputs):
    logq, host_const = kernel_logq(**inputs)
    total = float(np.mean(logq.astype(np.float64))) + host_const
    return np.float32(-total)
